# revision 1
# baseline (speedup 1.0000x reference)
"""Causal multi-head attention block (B=2, S=2048, D=768, H=12) on 8 trn2 cores.

Sharding: core c -> batch b = c//4 (data parallel), head group g = c%4
(tensor parallel, 3 heads per group). Each core computes its group's QKV
projection, causal attention, and a partial O-projection over its 192
z-columns. Host sums the 4 partials per batch and adds the biases that
commute through the math (v-bias and b_o).

On-core layout (everything "transposed", d on partitions, seq on free):
  xT   [768, 2048]   q/kT  [64*, 2048]      scores^T [keys, q]
so the softmax denominator comes free from a ones-column appended to V in
the PV matmul, and no on-chip transposes of activations are needed except
V (built via PE transpose from V^T).

The QKV projection uses a host-repacked weight matrix so every 128-wide
M-group is fully used:
  m0=[q_h0 q_h1] m1=[q_h2 v_h0] m2=[k_h0 k_h1] m3=[k_h2 v_h1] m4=[v_h2]
(q rows pre-scaled by 1/8; v bias folded into the host-side epilogue).

Matmul operands are float32r (full-rate fp32 on the PE). Scheduling
interleaves projection/transpose/O-proj work into the attention loop so
the scalar engine (exp) is never starved by a long PE FIFO stretch.
"""

import os
from collections import deque
from contextlib import ExitStack

import numpy as np

import concourse.tile as tile
from concourse import bacc, mybir
from concourse.bass_utils import run_bass_kernel_spmd
from concourse.masks import make_identity

F32 = mybir.dt.float32
F32R = mybir.dt.float32r
AF = mybir.ActivationFunctionType

B, S, D = 2, 2048, 768
NH, DH = 12, 64
HPC = 3            # heads per core
GD = HPC * DH      # 192 z-cols per core
KT, QT = 128, 512  # key tile (partitions), q tile (psum free)
NKT, NQT = S // KT, S // QT   # 16, 4
NTOK = S // 128    # 16 token tiles
NKD = D // 128     # 6 contraction tiles for the projections
WPK = 2 * GD + GD  # 576 packed projection rows


def build_bass():
    nc = bacc.Bacc(None)
    xT = nc.dram_tensor("xT", [D, S], F32, kind="ExternalInput")
    wpk = nc.dram_tensor("wpk", [D, WPK], F32, kind="ExternalInput")
    woT = nc.dram_tensor("woT", [GD, D], F32, kind="ExternalInput")
    bqk = nc.dram_tensor("bqk", [128, 4], F32, kind="ExternalInput")
    vones = nc.dram_tensor("vones", [128, 64], F32, kind="ExternalInput")
    out_p = nc.dram_tensor("out_p", [S, D], F32, kind="ExternalOutput")

    with tile.TileContext(nc) as tc, ExitStack() as ctx:
        const = ctx.enter_context(tc.tile_pool(name="const", bufs=1))
        ps = ctx.enter_context(tc.tile_pool(name="ps", bufs=6, space="PSUM"))
        psz = ctx.enter_context(tc.tile_pool(name="psz", bufs=2, space="PSUM"))
        expp = ctx.enter_context(tc.tile_pool(name="expp", bufs=9))
        small = ctx.enter_context(tc.tile_pool(name="small", bufs=4))

        xT_sb = const.tile([128, NKD, S], F32R)
        wpk_sb = const.tile([128, NKD, WPK], F32R)
        wo_a = const.tile([128, D], F32R)
        wo_b = const.tile([64, D], F32R)
        bqk_sb = const.tile([128, 4], F32)
        qT_sb = const.tile([128, 2, S], F32R)
        kT_sb = const.tile([128, 2, S], F32R)
        vvT = const.tile([128, 2, S], F32)
        v_aug = const.tile([128, HPC, NKT, DH + 1], F32R)
        zT01 = const.tile([128, S], F32R)
        zT2 = const.tile([64, S], F32R)
        ident = const.tile([128, 128], F32)
        ones64 = const.tile([1, 64], F32R)

        ones_stage = const.tile([128, 64], F32)
        make_identity(nc, ident[:])

        # ---- loads: k-interleaved so the first projection k-pairs unblock
        # early; everything not needed for (h0, qt0) comes after.
        xT_t = xT.rearrange("(t p) s -> t p s", p=128)
        wpk_t = wpk.rearrange("(t p) m -> t p m", p=128)
        for t in range(NKD):
            nc.sync.dma_start(
                out=wpk_sb[:, t, 0:384], in_=wpk_t[t][:, 0:384].bitcast(F32R)
            )
            nc.sync.dma_start(
                out=xT_sb[:, t, 0:QT], in_=xT_t[t][:, 0:QT].bitcast(F32R)
            )
        nc.sync.dma_start(out=bqk_sb[:], in_=bqk[:, :])
        for t in range(NKD):
            nc.sync.dma_start(
                out=wpk_sb[:, t, 384:WPK], in_=wpk_t[t][:, 384:WPK].bitcast(F32R)
            )
        nc.sync.dma_start(out=ones_stage[:], in_=vones[:, :])
        nc.sync.dma_start(out=ones64[:], in_=vones[0:1, 0:64].bitcast(F32R))
        nc.vector.tensor_copy(
            out=v_aug[:, :, :, DH],
            in_=ones_stage[:, 0 : HPC * NKT]
            .rearrange("p (h t) -> p h t", h=HPC)
            .bitcast(F32R),
        )
        for t in range(NKD):
            nc.sync.dma_start(
                out=xT_sb[:, t, QT : 2 * QT], in_=xT_t[t][:, QT : 2 * QT].bitcast(F32R)
            )
        nc.sync.dma_start(out=wo_a[:], in_=woT[0:128, :].bitcast(F32R))
        nc.sync.dma_start(out=wo_b[:], in_=woT[128:GD, :].bitcast(F32R))
        for t in range(NKD):
            nc.sync.dma_start(
                out=xT_sb[:, t, 2 * QT : S], in_=xT_t[t][:, 2 * QT : S].bitcast(F32R)
            )

        # packed projection m-groups: (col0, rows, evict spec)
        # evict spec: list of (psum row range, dst ap fn, bias col or None)
        def ev_q(col):
            return lambda n, r0, r1: qT_sb[r0:r1, col, n * QT : (n + 1) * QT]

        def ev_k(col):
            return lambda n, r0, r1: kT_sb[r0:r1, col, n * QT : (n + 1) * QT]

        def ev_v(col):
            return lambda n, r0, r1: vvT[r0:r1, col, n * QT : (n + 1) * QT]

        mgroups = [
            (0, 128, [((0, 128), ev_q(0), 0)]),
            (128, 128, [((0, 64), ev_q(1), 1), ((64, 128), ev_v(0), None)]),
            (256, 128, [((0, 128), ev_k(0), 2)]),
            (384, 128, [((0, 64), ev_k(1), 3), ((64, 128), ev_v(1), None)]),
            (512, 64, [((0, 64), ev_v(0), None)]),
        ]
        # v pieces: v_h0 -> vvT[64:128, 0], v_h1 -> vvT[64:128, 1],
        # v_h2 -> vvT[0:64, 0] (from the m4 group, psum rows 0:64)

        proj_psums = {}

        def proj_unit(mi, n, kpair):
            """Two K-step matmuls of group (mi, n); evictions after the last."""
            c0, msz, evicts = mgroups[mi]
            key = (mi, n)
            if key not in proj_psums:
                proj_psums[key] = ps.tile([128, QT], F32, tag="ps", name="projp")
            p = proj_psums[key]
            for k in (2 * kpair, 2 * kpair + 1):
                nc.tensor.matmul(
                    p[:msz, :],
                    lhsT=wpk_sb[:, k, c0 : c0 + msz],
                    rhs=xT_sb[:, k, n * QT : (n + 1) * QT],
                    start=(k == 0),
                    stop=(k == NKD - 1),
                )
            if kpair == 2:
                del proj_psums[key]
                for (r0, r1), dst, bcol in evicts:
                    if mi == 4:
                        dst_ap = dst(n, 0, 64)  # v_h2 rows live at psum 0:64
                    else:
                        dst_ap = dst(n, r0, r1)
                    if bcol is None:
                        nc.vector.tensor_copy(out=dst_ap, in_=p[r0:r1, :])
                    else:
                        nc.vector.tensor_scalar_add(
                            out=dst_ap,
                            in0=p[r0:r1, :],
                            scalar1=bqk_sb[r0:r1, bcol : bcol + 1],
                        )

        def transpose_unit(t, piece):
            """piece 0/1/2 = head 0/1/2; v_h0/v_h1 at vvT[64:128,0/1], v_h2 at vvT[0:64,0]."""
            if piece == 2:
                src = vvT[0:64, 0, t * 128 : (t + 1) * 128]
                idn = ident[0:64, 0:64]
            else:
                src = vvT[64:128, piece, t * 128 : (t + 1) * 128]
                idn = ident[64:128, 64:128]
            pt = ps.tile([128, QT], F32, tag="ps")
            nc.tensor.transpose(pt[:, 0:64], src, idn)
            nc.vector.tensor_copy(v_aug[:, piece, t, 0:64], pt[:, 0:64])

        out_pair = out_p.rearrange("(tp a p) d -> tp p a d", a=2, p=128)
        o_pairs = {}

        def o_proj_unit(t, n2):
            key = t // 2
            if key not in o_pairs:
                o_pairs[key] = expp.tile([128, 2, D], F32, tag="osb", name="osb", bufs=2)
            ob = o_pairs[key]
            po = ps.tile([128, QT], F32, tag="ps")
            nc.tensor.matmul(
                po[:, 0:384],
                lhsT=zT01[:, t * 128 : (t + 1) * 128],
                rhs=wo_a[:, n2 * 384 : (n2 + 1) * 384],
                start=True,
                stop=False,
            )
            nc.tensor.matmul(
                po[:, 0:384],
                lhsT=zT2[:, t * 128 : (t + 1) * 128],
                rhs=wo_b[:, n2 * 384 : (n2 + 1) * 384],
                start=False,
                stop=True,
            )
            if t >= 12 and (t + n2) % 2 == 0:
                nc.scalar.activation(
                    out=ob[:, t % 2, n2 * 384 : (n2 + 1) * 384],
                    in_=po[:, 0:384],
                    func=AF.Copy,
                )
            else:
                nc.vector.tensor_copy(
                    out=ob[:, t % 2, n2 * 384 : (n2 + 1) * 384], in_=po[:, 0:384]
                )
            if t % 2 == 1 and n2 == 1:
                del o_pairs[key]
                nc.sync.dma_start(out=out_pair[key], in_=ob[:, :, :])

        # background work queue of (key, fn), drained between attention
        # iterations. Queue order is topological (a group's transposes come
        # after its evictions), so force-draining "through the last needed
        # unit" preserves all producer->consumer program ordering.
        work = deque()

        def q_proj(n, mis=range(5)):
            for mi in mis:
                for kpair in range(3):
                    work.append(
                        (("proj", n, mi), lambda mi=mi, n=n, kp=kpair: proj_unit(mi, n, kp))
                    )

        def q_tr(ts, pieces=range(HPC)):
            for t in ts:
                for piece in pieces:
                    work.append(
                        (("tr", t, piece), lambda t=t, p=piece: transpose_unit(t, p))
                    )

        def drain(k=1):
            for _ in range(k):
                if work:
                    work.popleft()[1]()

        def drain_all():
            while work:
                work.popleft()[1]()

        PROJ_GROUPS_FOR_HEAD = {0: (0, 1, 2), 1: (0, 2, 3), 2: (1, 3, 4)}

        def force_drain_for(h, qt):
            """Emit queued units up to the last one attention(h, qt) depends on."""
            needed = set()
            for n in range(qt + 1):
                for mi in PROJ_GROUPS_FOR_HEAD[h]:
                    needed.add(("proj", n, mi))
            for t in range(4 * qt + 4):
                needed.add(("tr", t, h))
            last = -1
            for i, (key, _) in enumerate(work):
                if key in needed:
                    last = i
            for _ in range(last + 1):
                work.popleft()[1]()

        def qh(h):
            m, off = divmod(h * 64, 128)
            return qT_sb[off : off + 64, m, :]

        def kh(h):
            m, off = divmod(h * 64, 128)
            return kT_sb[off : off + 64, m, :]

        zdst = [zT01[0:64, :], zT01[64:128, :], zT2[0:64, :]]

        # PV matmuls are pipelined ~4 iterations behind their exp across
        # block boundaries, so the in-order PE FIFO never waits on the
        # exp/mask chain, not even at the end of a block.
        pvq = deque()  # (block_serial, pv_closure)
        blk_serial = [0]

        def pv_drain(depth):
            while len(pvq) > depth:
                pvq.popleft()[1]()

        def pv_flush(upto_serial):
            while pvq and pvq[0][0] <= upto_serial:
                pvq.popleft()[1]()

        def attention(h, qt, per_kt):
            """scores^T -> exp -> causal mask -> PV into zp; diagonal blocks
            narrowed to q columns >= 128*r."""
            zp = psz.tile([DH + 1, QT], F32)
            nkt = 4 * qt + 4
            blk = blk_serial[0]
            blk_serial[0] += 1

            def pv(kt, es, lo):
                nc.tensor.matmul(
                    zp[:, lo:QT],
                    lhsT=v_aug[:, h, kt, :],
                    rhs=es[:, lo:QT],
                    start=(kt == 0),
                    stop=(kt == nkt - 1),
                )

            for kt in range(nkt):
                rr = kt - 4 * qt
                lo = 128 * rr if rr > 0 else 0
                sp = ps.tile([128, QT], F32, tag="ps")
                nc.tensor.matmul(
                    sp[:, lo:QT],
                    lhsT=kh(h)[:, kt * 128 : (kt + 1) * 128],
                    rhs=qh(h)[:, qt * QT + lo : (qt + 1) * QT],
                    start=True,
                    stop=True,
                )
                es = expp.tile([128, QT], F32R, tag="expp")
                nc.scalar.activation(out=es[:, lo:QT], in_=sp[:, lo:QT], func=AF.Exp)
                if rr >= 0:  # diagonal block: zero where key > query
                    nc.gpsimd.affine_select(
                        out=es[:, lo:QT],
                        in_=es[:, lo:QT],
                        compare_op=mybir.AluOpType.is_ge,
                        fill=0.0,
                        base=0,
                        channel_multiplier=-1,
                        pattern=[[1, QT - lo]],
                    )
                pvq.append((blk, lambda kt=kt, es=es, lo=lo: pv(kt, es, lo)))
                if per_kt == 2:
                    drain(2)
                elif per_kt == 9:
                    drain(1)
                elif kt % 2 == 0:
                    drain(1)
                pv_drain(7)
            return zp, blk

        def normalize(zp, h, qt, cols=slice(0, QT)):
            rec = small.tile([1, QT], F32R, tag="rec")
            with nc.allow_low_precision(reason="f32r is fp32-precision"):
                nc.vector.reciprocal(rec[:, cols], zp[DH : DH + 1, cols])
            bc = ps.tile([128, QT], F32, tag="ps")
            nc.tensor.matmul(
                bc[0:64, cols], lhsT=ones64[:], rhs=rec[:, cols], start=True, stop=True
            )
            bc_sb = small.tile([64, QT], F32, tag="bcsb")
            if qt == NQT - 1:
                nc.vector.tensor_copy(out=bc_sb[:, cols], in_=bc[0:64, cols])
            else:
                nc.scalar.activation(out=bc_sb[:, cols], in_=bc[0:64, cols], func=AF.Copy)
            nc.vector.tensor_mul(
                zdst[h][:, qt * QT : (qt + 1) * QT][:, cols],
                zp[0:DH, cols],
                bc_sb[:, cols],
            )

        # ---- schedule ----
        # prologue: only what attention(h0, qt0) needs; the rest queues up.
        for mi in (0, 2, 1):
            for kpair in range(3):
                proj_unit(mi, 0, kpair)
        for t in range(4):
            transpose_unit(t, 0)
        q_proj(0, mis=(3,))
        q_tr(range(4), pieces=(1,))
        q_proj(0, mis=(4,))
        q_tr(range(4), pieces=(2,))
        for n in range(1, NQT):
            q_proj(n, mis=(0, 2, 1))
            q_tr(range(4 * n, 4 * n + 4), pieces=(0,))
            q_proj(n, mis=(3,))
            q_tr(range(4 * n, 4 * n + 4), pieces=(1,))
            q_proj(n, mis=(4,))
            q_tr(range(4 * n, 4 * n + 4), pieces=(2,))

        pending = None
        for qt in range(NQT):
            per_kt = [2, 1, 1, 9][qt]
            for h in range(HPC):
                force_drain_for(h, qt)
                zp, blk = attention(h, qt, per_kt)
                if pending is not None:
                    pv_flush(pending[3])  # pending block's PV accumulation done
                    normalize(*pending[:3])
                    ph, pqt = pending[1], pending[2]
                    if ph == HPC - 1:  # whole q-tile normalized -> O-proj ready
                        for t in range(4 * pqt, 4 * pqt + 4):
                            for n2 in range(2):
                                work.append(
                                    (("o", pqt), lambda t=t, n2=n2: o_proj_unit(t, n2))
                                )
                pending = (zp, h, qt, blk)
        # final block: normalize in column halves so the last O-proj pairs
        # start while the second half's recip/broadcast chain is still running
        pv_flush(pending[3])
        drain_all()
        normalize(*pending[:3], cols=slice(0, QT // 2))
        for t in (12, 13):
            for n2 in range(2):
                o_proj_unit(t, n2)
        normalize(*pending[:3], cols=slice(QT // 2, QT))
        for t in (14, 15):
            for n2 in range(2):
                o_proj_unit(t, n2)
    nc.finalize()
    return nc


_NC_CACHE = {}


def make_in_maps(x, W_qkv, b_qkv, W_o):
    in_maps = []
    for c in range(8):
        b, g = divmod(c, 4)
        hs = [HPC * g + i for i in range(HPC)]
        qr = [np.arange(64 * h, 64 * h + 64) for h in hs]
        w_q = [W_qkv[i] * 0.125 for i in qr]
        w_k = [W_qkv[768 + i] for i in qr]
        w_v = [W_qkv[1536 + i] for i in qr]
        b_q = [b_qkv[i] * 0.125 for i in qr]
        b_k = [b_qkv[768 + i] for i in qr]
        # packed rows: m0=[q0 q1] m1=[q2 v0] m2=[k0 k1] m3=[k2 v1] m4=[v2]
        wpk = np.concatenate(
            [w_q[0], w_q[1], w_q[2], w_v[0], w_k[0], w_k[1], w_k[2], w_v[1], w_v[2]],
            axis=0,
        )
        bqk_col = np.zeros((128, 4), np.float32)
        bqk_col[:, 0] = np.concatenate([b_q[0], b_q[1]])
        bqk_col[0:64, 1] = b_q[2]
        bqk_col[:, 2] = np.concatenate([b_k[0], b_k[1]])
        bqk_col[0:64, 3] = b_k[2]
        in_maps.append(
            {
                "xT": np.ascontiguousarray(x[b].T),
                "wpk": np.ascontiguousarray(wpk.T),
                "woT": np.ascontiguousarray(W_o[:, GD * g : GD * (g + 1)].T),
                "bqk": bqk_col,
                "vones": np.ones((128, 64), np.float32),
            }
        )
    return in_maps


def make_in_maps_for_test(inputs):
    return make_in_maps(
        np.asarray(inputs["x"], np.float32),
        np.asarray(inputs["W_qkv"], np.float32),
        np.asarray(inputs["b_qkv"], np.float32),
        np.asarray(inputs["W_o"], np.float32),
    )


def kernel(x, W_qkv, b_qkv, W_o, b_o):
    x = np.asarray(x, np.float32)
    W_qkv = np.asarray(W_qkv, np.float32)
    b_qkv = np.asarray(b_qkv, np.float32)
    W_o = np.asarray(W_o, np.float32)
    b_o = np.asarray(b_o, np.float32)

    if "nc" not in _NC_CACHE:
        _NC_CACHE["nc"] = build_bass()
    nc = _NC_CACHE["nc"]

    in_maps = make_in_maps(x, W_qkv, b_qkv, W_o)

    res = run_bass_kernel_spmd(
        nc,
        in_maps,
        list(range(8)),
        trace=bool(int(os.environ.get("KERNEL_TRACE", "0"))),
    )
    _NC_CACHE["last_results"] = res

    out = np.zeros((B, S, D), np.float32)
    for c in range(8):
        out[c // 4] += res.results[c]["out_p"]
    out += b_qkv[1536:] @ W_o.T + b_o
    return out



# revision 25
# speedup vs baseline: 1.0618x; 1.0618x over previous
"""Causal multi-head attention block (B=2, S=2048, D=768, H=12) on 8 trn2 cores.

Sharding: core c -> batch b = c//4 (data parallel), head group g = c%4
(tensor parallel, 3 heads per group). Each core computes its group's QKV
projection, causal attention, and a partial O-projection over its 192
z-columns. Host sums the 4 partials per batch and adds the biases that
commute through the math (v-bias and b_o).

On-core layout (everything "transposed", d on partitions, seq on free):
  xT   [768, 2048]   q/kT  [64*, 2048]      scores^T [keys, q]
so the softmax denominator comes free from a ones-column appended to V in
the PV matmul, and no on-chip transposes of activations are needed except
V (built via PE transpose from V^T).

The QKV projection uses a host-repacked weight matrix so every 128-wide
M-group is fully used:
  m0=[q_h0 q_h1] m1=[q_h2 v_h0] m2=[k_h0 k_h1] m3=[k_h2 v_h1] m4=[v_h2]
(q rows pre-scaled by 1/8; v bias folded into the host-side epilogue).

Matmul operands are float32r (full-rate fp32 on the PE). Scheduling
interleaves projection/transpose/O-proj work into the attention loop so
the scalar engine (exp) is never starved by a long PE FIFO stretch.

Cost-structure details:
  - fp32r matmuls under 256 output columns run at 1/4 rate, so the rr=3
    diagonal block is computed 256 wide (the causal mask zeroes the
    overhang) instead of 128 wide.
  - causal masks only touch the 128/256-wide zone that can violate
    causality instead of the whole remaining tile.
  - V^T and the identity live in f32r so the V transposes run at 1.5
    cycles/row instead of fp32's 2.0.
  - dummy PE transposes during the DMA prologue keep the tensor engine
    busy so the p-state ramp finishes before real matmuls arrive.
  - the last four token tiles store per 384-column half as soon as each
    O-proj eviction lands, shortening the end-of-kernel DMA tail.
"""

import os
from collections import deque
from contextlib import ExitStack

import numpy as np
import ml_dtypes

import concourse.tile as tile
from concourse import bacc, mybir
from concourse.bass_utils import run_bass_kernel_spmd

F32 = mybir.dt.float32
F32R = mybir.dt.float32r
AF = mybir.ActivationFunctionType

B, S, D = 2, 2048, 768
NH, DH = 12, 64
HPC = 3            # heads per core
GD = HPC * DH      # 192 z-cols per core
KT, QT = 128, 512  # key tile (partitions), q tile (psum free)
NKT, NQT = S // KT, S // QT   # 16, 4
NTOK = S // 128    # 16 token tiles
NKD = D // 128     # 6 contraction tiles for the projections
WPK = 2 * GD + GD  # 576 packed projection rows

N_WARMUP = int(os.environ.get("K_WARMUP", "16"))   # dummy PE transposes in prologue
PVQ = int(os.environ.get("K_PVQ", "7"))            # PV queue lag depth
LPVQ = int(os.environ.get("K_LPVQ", "7"))          # PV lag depth for the final block
FCOPY = os.environ.get("K_FCOPY", "mix")           # act|mix: final 4-tile eviction engines
NORM_EARLY = os.environ.get("K_NORME", "0") == "1" # normalize before next force_drain
PACE3 = int(os.environ.get("K_PACE3", "9"))        # qt3 drain mode
BF16_IN = os.environ.get("K_BF16", "1") == "1"     # load x / W_qkv in bf16
BF16_OUT = os.environ.get("K_BF16O", "1") == "1"   # store O-proj partials in bf16
EV_SPLIT = os.environ.get("K_EVSPLIT", "none")   # none|pool2|pool23: proj evictions offload
PACE0 = int(os.environ.get("K_PACE0", "2"))        # qt0 drain units per kt
EPI = os.environ.get("K_EPI", "half")           # half|quarter: final normalize granularity


def build_bass():
    nc = bacc.Bacc(None)
    in_dt_d = mybir.dt.bfloat16 if BF16_IN else F32
    xT = nc.dram_tensor("xT", [D, S], in_dt_d, kind="ExternalInput")
    wpk = nc.dram_tensor("wpk", [D, WPK], in_dt_d, kind="ExternalInput")
    woT = nc.dram_tensor("woT", [GD, D], F32, kind="ExternalInput")
    bqk = nc.dram_tensor("bqk", [128, 4], F32, kind="ExternalInput")
    vones = nc.dram_tensor("vones", [128, 64], F32, kind="ExternalInput")
    out_dt_d = mybir.dt.bfloat16 if BF16_OUT else F32
    out_p = nc.dram_tensor("out_p", [S, D], out_dt_d, kind="ExternalOutput")

    with tile.TileContext(nc) as tc, ExitStack() as ctx:
        const = ctx.enter_context(tc.tile_pool(name="const", bufs=1))
        ps = ctx.enter_context(tc.tile_pool(name="ps", bufs=6, space="PSUM"))
        psz = ctx.enter_context(tc.tile_pool(name="psz", bufs=2, space="PSUM"))
        expp = ctx.enter_context(tc.tile_pool(name="expp", bufs=9))
        small = ctx.enter_context(tc.tile_pool(name="small", bufs=4))

        IN_DT = mybir.dt.bfloat16 if BF16_IN else F32R
        OUT_DT = mybir.dt.bfloat16 if BF16_OUT else F32
        xT_sb = const.tile([128, NKD, S], IN_DT)
        wpk_sb = const.tile([128, NKD, WPK], IN_DT)
        wo_a = const.tile([128, D], F32R)
        wo_b = const.tile([64, D], F32R)
        bqk_sb = const.tile([128, 4], F32)
        qT_sb = const.tile([128, 2, S], F32R)
        kT_sb = const.tile([128, 2, S], F32R)
        vvT = const.tile([128, 2, S], F32R)
        v_aug = const.tile([128, HPC, NKT, DH + 1], F32R)
        zT01 = const.tile([128, S], F32R)
        zT2 = const.tile([64, S], F32R)
        ident = const.tile([128, 128], F32R)
        ones64 = const.tile([1, 64], F32R)

        # warmup: keep the PE busy while the first DMAs land so the p-state
        # ramp is done before real matmuls issue. The transposes read the
        # not-yet-written identity tile; the values are irrelevant (nothing
        # reads `warm`) and the WAR ordering only delays make_identity to
        # ~1.5us, well before the first real V transpose needs it.
        warm = ps.tile([128, QT], F32R, tag="ps", name="warm")
        for _ in range(N_WARMUP):
            nc.tensor.transpose(warm[:, 0:128], ident[:, :], ident[:, :])

        # f32r identity: memset must run on an f32 view (Memset of f32r
        # fails the ISA check) while the affine_select writes the f32r view
        # so downstream f32r matmuls see properly rounded inputs.
        nc.gpsimd.memset(ident[:].bitcast(F32), 0.0)
        nc.gpsimd.affine_select(
            out=ident[:],
            in_=ident[:],
            compare_op=mybir.AluOpType.not_equal,
            fill=1.0,
            base=0,
            pattern=[[-1, 128]],
            channel_multiplier=1,
        )
        ones_stage = const.tile([128, 64], F32)
        nc.sync.dma_start(out=ones_stage[:], in_=vones[:, :])
        nc.sync.dma_start(out=ones64[:], in_=vones[0:1, 0:64].bitcast(F32R))
        nc.vector.tensor_copy(
            out=v_aug[:, :, :, DH],
            in_=ones_stage[:, 0 : HPC * NKT]
            .rearrange("p (h t) -> p h t", h=HPC)
            .bitcast(F32R),
        )

        # ---- loads. HWDGE costs ~625ns of descriptor generation per
        # dma_start regardless of size, so tiles are fetched in k-PAIRS for
        # the latency-critical prologue (projection k-steps consume pairs)
        # and in bigger merged transfers for everything later.
        xT_p = xT.rearrange("(t p) s -> p t s", p=128)
        wpk_p = wpk.rearrange("(t p) m -> p t m", p=128)
        for t0 in range(0, NKD, 2):
            nc.sync.dma_start(
                out=wpk_sb[:, t0 : t0 + 2, 0:384], in_=wpk_p[:, t0 : t0 + 2, 0:384]
            )
            nc.sync.dma_start(
                out=xT_sb[:, t0 : t0 + 2, 0:QT], in_=xT_p[:, t0 : t0 + 2, 0:QT]
            )
        nc.sync.dma_start(out=bqk_sb[:], in_=bqk[:, :])
        nc.sync.dma_start(out=wpk_sb[:, :, 384:WPK], in_=wpk_p[:, :, 384:WPK])
        for t0 in range(0, NKD, 3):
            nc.sync.dma_start(
                out=xT_sb[:, t0 : t0 + 3, QT : 2 * QT],
                in_=xT_p[:, t0 : t0 + 3, QT : 2 * QT],
            )
        nc.sync.dma_start(out=wo_a[:], in_=woT[0:128, :].bitcast(F32R))
        nc.sync.dma_start(out=wo_b[:], in_=woT[128:GD, :].bitcast(F32R))
        for t0 in range(0, NKD, 3):
            nc.sync.dma_start(
                out=xT_sb[:, t0 : t0 + 3, 2 * QT : S],
                in_=xT_p[:, t0 : t0 + 3, 2 * QT : S],
            )

        # packed projection m-groups: (col0, rows, evict spec)
        # evict spec: list of (psum row range, dst ap fn, bias col or None)
        def ev_q(col):
            return lambda n, r0, r1: qT_sb[r0:r1, col, n * QT : (n + 1) * QT]

        def ev_k(col):
            return lambda n, r0, r1: kT_sb[r0:r1, col, n * QT : (n + 1) * QT]

        def ev_v(col):
            return lambda n, r0, r1: vvT[r0:r1, col, n * QT : (n + 1) * QT]

        mgroups = [
            (0, 128, [((0, 128), ev_q(0), 0)]),
            (128, 128, [((0, 64), ev_q(1), 1), ((64, 128), ev_v(0), None)]),
            (256, 128, [((0, 128), ev_k(0), 2)]),
            (384, 128, [((0, 64), ev_k(1), 3), ((64, 128), ev_v(1), None)]),
            (512, 64, [((0, 64), ev_v(0), None)]),
        ]
        # v pieces: v_h0 -> vvT[64:128, 0], v_h1 -> vvT[64:128, 1],
        # v_h2 -> vvT[0:64, 0] (from the m4 group, psum rows 0:64)

        proj_psums = {}

        def proj_unit(mi, n, kpair):
            """Two K-step matmuls of group (mi, n); evictions after the last."""
            c0, msz, evicts = mgroups[mi]
            key = (mi, n)
            if key not in proj_psums:
                proj_psums[key] = ps.tile([128, QT], F32, tag="ps", name="projp")
            p = proj_psums[key]
            for k in (2 * kpair, 2 * kpair + 1):
                nc.tensor.matmul(
                    p[:msz, :],
                    lhsT=wpk_sb[:, k, c0 : c0 + msz],
                    rhs=xT_sb[:, k, n * QT : (n + 1) * QT],
                    start=(k == 0),
                    stop=(k == NKD - 1),
                )
            if kpair == 2:
                del proj_psums[key]
                for (r0, r1), dst, bcol in evicts:
                    if mi == 4:
                        dst_ap = dst(n, 0, 64)  # v_h2 rows live at psum 0:64
                    else:
                        dst_ap = dst(n, r0, r1)
                    if bcol is None:
                        nc.vector.tensor_copy(out=dst_ap, in_=p[r0:r1, :])
                        continue
                    # NOTE: GPSIMD/Pool cannot read PSUM, so eviction
                    # offload is limited to ACT (Identity = copy + bias).
                    if (EV_SPLIT == "act0" and mi == 0) or (
                        EV_SPLIT == "act02" and mi in (0, 2)
                    ):
                        nc.scalar.activation(
                            out=dst_ap,
                            in_=p[r0:r1, :],
                            func=AF.Identity,
                            bias=bqk_sb[r0:r1, bcol : bcol + 1],
                        )
                    else:
                        nc.vector.tensor_scalar_add(
                            out=dst_ap,
                            in0=p[r0:r1, :],
                            scalar1=bqk_sb[r0:r1, bcol : bcol + 1],
                        )

        def transpose_unit(t, piece):
            """piece 0/1/2 = head 0/1/2; v_h0/v_h1 at vvT[64:128,0/1], v_h2 at vvT[0:64,0]."""
            if piece == 2:
                src = vvT[0:64, 0, t * 128 : (t + 1) * 128]
                idn = ident[0:64, 0:64]
            else:
                src = vvT[64:128, piece, t * 128 : (t + 1) * 128]
                idn = ident[64:128, 64:128]
            pt = ps.tile([128, QT], F32R, tag="ps")
            nc.tensor.transpose(pt[:, 0:64], src, idn)
            nc.vector.tensor_copy(v_aug[:, piece, t, 0:64], pt[:, 0:64])

        out_pair = out_p.rearrange("(tp a p) d -> tp p a d", a=2, p=128)
        out_sing = out_p.rearrange("(t p) d -> t p d", p=128)
        o_pairs = {}

        def o_proj_unit(t, n2, solo_dma=False):
            key = t // 2
            po = ps.tile([128, QT], F32, tag="ps")
            nc.tensor.matmul(
                po[:, 0:384],
                lhsT=zT01[:, t * 128 : (t + 1) * 128],
                rhs=wo_a[:, n2 * 384 : (n2 + 1) * 384],
                start=True,
                stop=False,
            )
            nc.tensor.matmul(
                po[:, 0:384],
                lhsT=zT2[:, t * 128 : (t + 1) * 128],
                rhs=wo_b[:, n2 * 384 : (n2 + 1) * 384],
                start=False,
                stop=True,
            )
            if solo_dma:
                # end of kernel: copies alternate ACT/DVE so the two halves
                # stage in parallel, then one per-tile store fires.
                ob = o_pairs.setdefault(
                    key, expp.tile([128, 2, D], OUT_DT, tag="osb", name="osb", bufs=2)
                )
                dst = ob[:, t % 2, n2 * 384 : (n2 + 1) * 384]
                if FCOPY == "act" or n2 == 0:
                    nc.scalar.activation(out=dst, in_=po[:, 0:384], func=AF.Copy)
                else:
                    nc.vector.tensor_copy(out=dst, in_=po[:, 0:384])
                if n2 == 1:
                    nc.sync.dma_start(out=out_sing[t], in_=ob[:, t % 2, :])
                    if t % 2 == 1:
                        del o_pairs[key]
                return
            if key not in o_pairs:
                o_pairs[key] = expp.tile([128, 2, D], OUT_DT, tag="osb", name="osb", bufs=2)
            ob = o_pairs[key]
            if t >= 12 and (t + n2) % 2 == 0:
                nc.scalar.activation(
                    out=ob[:, t % 2, n2 * 384 : (n2 + 1) * 384],
                    in_=po[:, 0:384],
                    func=AF.Copy,
                )
            else:
                nc.vector.tensor_copy(
                    out=ob[:, t % 2, n2 * 384 : (n2 + 1) * 384], in_=po[:, 0:384]
                )
            if t % 2 == 1 and n2 == 1:
                del o_pairs[key]
                nc.sync.dma_start(out=out_pair[key], in_=ob[:, :, :])

        # background work queue of (key, fn), drained between attention
        # iterations. Queue order is topological (a group's transposes come
        # after its evictions), so force-draining "through the last needed
        # unit" preserves all producer->consumer program ordering.
        work = deque()

        def q_proj(n, mis=range(5)):
            for mi in mis:
                for kpair in range(3):
                    work.append(
                        (("proj", n, mi), lambda mi=mi, n=n, kp=kpair: proj_unit(mi, n, kp))
                    )

        trq = deque()  # transpose units, pulled only by ensure_tr (PV time)

        def q_tr(ts, pieces=range(HPC)):
            for t in ts:
                for piece in pieces:
                    trq.append(
                        ((t, piece), lambda t=t, p=piece: transpose_unit(t, p))
                    )

        def drain(k=1):
            for _ in range(k):
                if work:
                    work.popleft()[1]()

        def drain_all():
            while work:
                work.popleft()[1]()
            while trq:
                trq.popleft()[1]()

        PROJ_GROUPS_FOR_HEAD = {0: (0, 1, 2), 1: (0, 2, 3), 2: (1, 3, 4)}

        def _drain_through(needed):
            last = -1
            for i, (key, _) in enumerate(work):
                if key in needed:
                    last = i
            for _ in range(last + 1):
                work.popleft()[1]()

        def force_drain_for(h, qt):
            """Emit queued units the SCORES of attention(h, qt) depend on.
            V transposes are pulled lazily by the PV closures instead."""
            needed = set()
            for n in range(qt + 1):
                for mi in PROJ_GROUPS_FOR_HEAD[h]:
                    needed.add(("proj", n, mi))
            _drain_through(needed)

        def ensure_tr(t, piece):
            last = -1
            for i, (key, _) in enumerate(trq):
                if key == (t, piece):
                    last = i
            for _ in range(last + 1):
                trq.popleft()[1]()

        def qh(h):
            m, off = divmod(h * 64, 128)
            return qT_sb[off : off + 64, m, :]

        def kh(h):
            m, off = divmod(h * 64, 128)
            return kT_sb[off : off + 64, m, :]

        zdst = [zT01[0:64, :], zT01[64:128, :], zT2[0:64, :]]

        # PV matmuls are pipelined ~4 iterations behind their exp across
        # block boundaries, so the in-order PE FIFO never waits on the
        # exp/mask chain, not even at the end of a block.
        pvq = deque()  # (block_serial, pv_closure)
        blk_serial = [0]

        def pv_drain(depth):
            while len(pvq) > depth:
                pvq.popleft()[1]()

        def pv_flush(upto_serial):
            while pvq and pvq[0][0] <= upto_serial:
                pvq.popleft()[1]()

        def lo_of(rr):
            # query-column start of the computed region for a key tile with
            # diagonal offset rr; rr=3 is held at 256 so the fp32r matmuls
            # stay >=256 wide (narrower pays 4x in rate).
            if rr <= 0:
                return 0
            return 128 * rr if rr < 3 else 256

        def attention(h, qt, per_kt, pvdepth=None):
            """scores^T -> exp -> causal mask (narrow zone) -> PV into zp."""
            if pvdepth is None:
                pvdepth = PVQ
            zp = psz.tile([DH + 1, QT], F32)
            nkt = 4 * qt + 4
            blk = blk_serial[0]
            blk_serial[0] += 1

            def pv(kt, es, lo):
                ensure_tr(kt, h)
                nc.tensor.matmul(
                    zp[:, lo:QT],
                    lhsT=v_aug[:, h, kt, :],
                    rhs=es[:, lo:QT],
                    start=(kt == 0),
                    stop=(kt == nkt - 1),
                )

            for kt in range(nkt):
                rr = kt - 4 * qt
                lo = lo_of(rr)
                sp = ps.tile([128, QT], F32, tag="ps")
                nc.tensor.matmul(
                    sp[:, lo:QT],
                    lhsT=kh(h)[:, kt * 128 : (kt + 1) * 128],
                    rhs=qh(h)[:, qt * QT + lo : (qt + 1) * QT],
                    start=True,
                    stop=True,
                )
                es = expp.tile([128, QT], F32R, tag="expp")
                nc.scalar.activation(out=es[:, lo:QT], in_=sp[:, lo:QT], func=AF.Exp)
                if rr >= 0:  # diagonal: zero where key > query
                    z0 = 128 * rr if rr < 3 else 256
                    zw = 128 if rr < 3 else 256
                    nc.gpsimd.affine_select(
                        out=es[:, z0 : z0 + zw],
                        in_=es[:, z0 : z0 + zw],
                        compare_op=mybir.AluOpType.is_ge,
                        fill=0.0,
                        base=z0 - 128 * rr,
                        channel_multiplier=-1,
                        pattern=[[1, zw]],
                    )
                pvq.append((blk, lambda kt=kt, es=es, lo=lo: pv(kt, es, lo)))
                if per_kt in (2, 3, 4):
                    drain(per_kt)
                elif per_kt == 9:
                    drain(1)
                elif kt % 2 == 0:
                    drain(1)
                pv_drain(pvdepth)
            return zp, blk

        def normalize(zp, h, qt, cols=slice(0, QT)):
            rec = small.tile([1, QT], F32R, tag="rec")
            with nc.allow_low_precision(reason="f32r is fp32-precision"):
                nc.vector.reciprocal(rec[:, cols], zp[DH : DH + 1, cols])
            bc = ps.tile([128, QT], F32, tag="ps")
            nc.tensor.matmul(
                bc[0:64, cols], lhsT=ones64[:], rhs=rec[:, cols], start=True, stop=True
            )
            bc_sb = small.tile([64, QT], F32, tag="bcsb")
            if qt == NQT - 1:
                nc.vector.tensor_copy(out=bc_sb[:, cols], in_=bc[0:64, cols])
            else:
                nc.scalar.activation(out=bc_sb[:, cols], in_=bc[0:64, cols], func=AF.Copy)
            nc.vector.tensor_mul(
                zdst[h][:, qt * QT : (qt + 1) * QT][:, cols],
                zp[0:DH, cols],
                bc_sb[:, cols],
            )

        # ---- schedule ----
        # prologue: only what attention(h0, qt0) needs; the rest queues up.
        for mi in (0, 2, 1):
            for kpair in range(3):
                proj_unit(mi, 0, kpair)
        q_tr(range(4), pieces=(0,))
        q_proj(0, mis=(3,))
        q_tr(range(4), pieces=(1,))
        q_proj(0, mis=(4,))
        q_tr(range(4), pieces=(2,))
        for n in range(1, NQT):
            q_proj(n, mis=(0, 2, 1))
            q_tr(range(4 * n, 4 * n + 4), pieces=(0,))
            q_proj(n, mis=(3,))
            q_tr(range(4 * n, 4 * n + 4), pieces=(1,))
            q_proj(n, mis=(4,))
            q_tr(range(4 * n, 4 * n + 4), pieces=(2,))

        pending = None
        for qt in range(NQT):
            per_kt = [PACE0, 1, 1, PACE3][qt]
            for h in range(HPC):
                if pending is not None and NORM_EARLY:
                    # normalize the pending block BEFORE the next block's
                    # eviction burst so its reciprocal isn't queued behind
                    # them on DVE (the PE-side broadcast waits on it)
                    pv_flush(pending[3])
                    normalize(*pending[:3])
                    ph, pqt = pending[1], pending[2]
                    if ph == HPC - 1:
                        for t in range(4 * pqt, 4 * pqt + 4):
                            for n2 in range(2):
                                work.append(
                                    (("o", pqt), lambda t=t, n2=n2: o_proj_unit(t, n2))
                                )
                    pending = None
                force_drain_for(h, qt)
                zp, blk = attention(
                    h, qt, per_kt,
                    pvdepth=LPVQ if (qt == NQT - 1 and h == HPC - 1) else PVQ,
                )
                if pending is not None:
                    pv_flush(pending[3])  # pending block's PV accumulation done
                    normalize(*pending[:3])
                    ph, pqt = pending[1], pending[2]
                    if ph == HPC - 1:  # whole q-tile normalized -> O-proj ready
                        for t in range(4 * pqt, 4 * pqt + 4):
                            for n2 in range(2):
                                work.append(
                                    (("o", pqt), lambda t=t, n2=n2: o_proj_unit(t, n2))
                                )
                pending = (zp, h, qt, blk)
        # final block: normalize in column halves so the last O-proj pairs
        # start while the second half's recip/broadcast chain is still running
        pv_flush(pending[3])
        drain_all()
        if EPI == "quarter":
            for quarter in range(4):
                normalize(*pending[:3], cols=slice(quarter * 128, (quarter + 1) * 128))
                for n2 in range(2):
                    o_proj_unit(12 + quarter, n2, solo_dma=True)
        else:
            for half in range(2):
                normalize(*pending[:3], cols=slice(half * 256, (half + 1) * 256))
                for t in (12 + 2 * half, 13 + 2 * half):
                    for n2 in range(2):
                        o_proj_unit(t, n2, solo_dma=True)
    nc.finalize()
    return nc


_NC_CACHE = {}


def make_in_maps(x, W_qkv, b_qkv, W_o):
    in_maps = []
    for c in range(8):
        b, g = divmod(c, 4)
        hs = [HPC * g + i for i in range(HPC)]
        qr = [np.arange(64 * h, 64 * h + 64) for h in hs]
        w_q = [W_qkv[i] * 0.125 for i in qr]
        w_k = [W_qkv[768 + i] for i in qr]
        w_v = [W_qkv[1536 + i] for i in qr]
        b_q = [b_qkv[i] * 0.125 for i in qr]
        b_k = [b_qkv[768 + i] for i in qr]
        # packed rows: m0=[q0 q1] m1=[q2 v0] m2=[k0 k1] m3=[k2 v1] m4=[v2]
        wpk = np.concatenate(
            [w_q[0], w_q[1], w_q[2], w_v[0], w_k[0], w_k[1], w_k[2], w_v[1], w_v[2]],
            axis=0,
        )
        bqk_col = np.zeros((128, 4), np.float32)
        bqk_col[:, 0] = np.concatenate([b_q[0], b_q[1]])
        bqk_col[0:64, 1] = b_q[2]
        bqk_col[:, 2] = np.concatenate([b_k[0], b_k[1]])
        bqk_col[0:64, 3] = b_k[2]
        in_dt = ml_dtypes.bfloat16 if os.environ.get("K_BF16", "1") == "1" else np.float32
        in_maps.append(
            {
                "xT": np.ascontiguousarray(x[b].T).astype(in_dt),
                "wpk": np.ascontiguousarray(wpk.T).astype(in_dt),
                "woT": np.ascontiguousarray(W_o[:, GD * g : GD * (g + 1)].T),
                "bqk": bqk_col,
                "vones": np.ones((128, 64), np.float32),
            }
        )
    return in_maps


def make_in_maps_for_test(inputs):
    return make_in_maps(
        np.asarray(inputs["x"], np.float32),
        np.asarray(inputs["W_qkv"], np.float32),
        np.asarray(inputs["b_qkv"], np.float32),
        np.asarray(inputs["W_o"], np.float32),
    )


def kernel(x, W_qkv, b_qkv, W_o, b_o):
    x = np.asarray(x, np.float32)
    W_qkv = np.asarray(W_qkv, np.float32)
    b_qkv = np.asarray(b_qkv, np.float32)
    W_o = np.asarray(W_o, np.float32)
    b_o = np.asarray(b_o, np.float32)

    if "nc" not in _NC_CACHE:
        _NC_CACHE["nc"] = build_bass()
    nc = _NC_CACHE["nc"]

    in_maps = make_in_maps(x, W_qkv, b_qkv, W_o)

    res = run_bass_kernel_spmd(
        nc,
        in_maps,
        list(range(8)),
        trace=bool(int(os.environ.get("KERNEL_TRACE", "0"))),
    )
    _NC_CACHE["last_results"] = res

    out = np.zeros((B, S, D), np.float32)
    for c in range(8):
        out[c // 4] += np.asarray(res.results[c]["out_p"], np.float32)
    out += b_qkv[1536:] @ W_o.T + b_o
    return out


# revision 29
# speedup vs baseline: 1.0745x; 1.0120x over previous
"""Causal multi-head attention block (B=2, S=2048, D=768, H=12) on 8 trn2 cores.

Sharding: core c -> batch b = c//4 (data parallel), head group g = c%4
(tensor parallel, 3 heads per group). Each core computes its group's QKV
projection, causal attention, and a partial O-projection over its 192
z-columns. Host sums the 4 partials per batch and adds the biases that
commute through the math (v-bias and b_o).

On-core layout (everything "transposed", d on partitions, seq on free):
  xT   [768, 2048]   q/kT  [64*, 2048]      scores^T [keys, q]
so the softmax denominator comes free from a ones-column appended to V in
the PV matmul, and no on-chip transposes of activations are needed except
V (built via PE transpose from V^T).

The QKV projection uses a host-repacked weight matrix so every 128-wide
M-group is fully used:
  m0=[q_h0 q_h1] m1=[q_h2 v_h0] m2=[k_h0 k_h1] m3=[k_h2 v_h1] m4=[v_h2]
(q rows pre-scaled by 1/8; v bias folded into the host-side epilogue).

Matmul operands are float32r (full-rate fp32 on the PE). Scheduling
interleaves projection/transpose/O-proj work into the attention loop so
the scalar engine (exp) is never starved by a long PE FIFO stretch.

Cost-structure details:
  - fp32r matmuls under 256 output columns run at 1/4 rate, so the rr=3
    diagonal block is computed 256 wide (the causal mask zeroes the
    overhang) instead of 128 wide.
  - causal masks only touch the 128/256-wide zone that can violate
    causality instead of the whole remaining tile.
  - V^T and the identity live in f32r so the V transposes run at 1.5
    cycles/row instead of fp32's 2.0.
  - dummy PE transposes during the DMA prologue keep the tensor engine
    busy so the p-state ramp finishes before real matmuls arrive.
  - the last four token tiles store per 384-column half as soon as each
    O-proj eviction lands, shortening the end-of-kernel DMA tail.
"""

import os
from collections import deque
from contextlib import ExitStack

import numpy as np
import ml_dtypes

import concourse.tile as tile
from concourse import bacc, mybir
from concourse.bass_utils import run_bass_kernel_spmd

F32 = mybir.dt.float32
F32R = mybir.dt.float32r
AF = mybir.ActivationFunctionType

B, S, D = 2, 2048, 768
NH, DH = 12, 64
HPC = 3            # heads per core
GD = HPC * DH      # 192 z-cols per core
KT, QT = 128, 512  # key tile (partitions), q tile (psum free)
NKT, NQT = S // KT, S // QT   # 16, 4
NTOK = S // 128    # 16 token tiles
NKD = D // 128     # 6 contraction tiles for the projections
WPK = 2 * GD + GD  # 576 packed projection rows

N_WARMUP = int(os.environ.get("K_WARMUP", "16"))   # dummy PE transposes in prologue
PVQ = int(os.environ.get("K_PVQ", "7"))            # PV queue lag depth
LPVQ = int(os.environ.get("K_LPVQ", "7"))          # PV lag depth for the final block
FCOPY = os.environ.get("K_FCOPY", "mix")           # act|mix: final 4-tile eviction engines
NORM_EARLY = os.environ.get("K_NORME", "0") == "1" # normalize before next force_drain
PACE3 = int(os.environ.get("K_PACE3", "9"))        # qt3 drain mode
BF16_IN = os.environ.get("K_BF16", "1") == "1"     # load x / W_qkv in bf16
BF16_OUT = os.environ.get("K_BF16O", "1") == "1"   # store O-proj partials in bf16
EV_SPLIT = os.environ.get("K_EVSPLIT", "none")   # none|pool2|pool23: proj evictions offload
PACE0 = int(os.environ.get("K_PACE0", "2"))        # qt0 drain units per kt
EPI = os.environ.get("K_EPI", "half")           # half|quarter: final normalize granularity


def build_bass():
    nc = bacc.Bacc(None)
    in_dt_d = mybir.dt.bfloat16 if BF16_IN else F32
    xT = nc.dram_tensor("xT", [D, S], in_dt_d, kind="ExternalInput")
    wpk = nc.dram_tensor("wpk", [D, WPK], in_dt_d, kind="ExternalInput")
    woT = nc.dram_tensor("woT", [GD, D], F32, kind="ExternalInput")
    bqk = nc.dram_tensor("bqk", [128, 4], F32, kind="ExternalInput")
    vones = nc.dram_tensor("vones", [128, 64], F32, kind="ExternalInput")
    out_dt_d = mybir.dt.bfloat16 if BF16_OUT else F32
    out_p = nc.dram_tensor("out_p", [S, D], out_dt_d, kind="ExternalOutput")

    with tile.TileContext(nc) as tc, ExitStack() as ctx:
        const = ctx.enter_context(tc.tile_pool(name="const", bufs=1))
        ps = ctx.enter_context(tc.tile_pool(name="ps", bufs=6, space="PSUM"))
        psz = ctx.enter_context(tc.tile_pool(name="psz", bufs=2, space="PSUM"))
        expp = ctx.enter_context(tc.tile_pool(name="expp", bufs=9))
        small = ctx.enter_context(tc.tile_pool(name="small", bufs=4))

        IN_DT = mybir.dt.bfloat16 if BF16_IN else F32R
        OUT_DT = mybir.dt.bfloat16 if BF16_OUT else F32
        xT_sb = const.tile([128, NKD, S], IN_DT)
        wpk_sb = const.tile([128, NKD, WPK], IN_DT)
        wo_a = const.tile([128, D], F32R)
        wo_b = const.tile([64, D], F32R)
        bqk_sb = const.tile([128, 4], F32)
        qT_sb = const.tile([128, 2, S], F32R)
        kT_sb = const.tile([128, 2, S], F32R)
        vvT = const.tile([128, 2, S], F32R)
        v_aug = const.tile([128, HPC, NKT, DH + 1], F32R)
        zT01 = const.tile([128, S], F32R)
        zT2 = const.tile([64, S], F32R)
        ident = const.tile([128, 128], F32R)
        ones64 = const.tile([1, 64], F32R)

        # warmup: keep the PE busy while the first DMAs land so the p-state
        # ramp is done before real matmuls issue. The transposes read the
        # not-yet-written identity tile; the values are irrelevant (nothing
        # reads `warm`) and the WAR ordering only delays make_identity to
        # ~1.5us, well before the first real V transpose needs it.
        warm = ps.tile([128, QT], F32R, tag="ps", name="warm")
        for _ in range(N_WARMUP):
            nc.tensor.transpose(warm[:, 0:128], ident[:, :], ident[:, :])

        # f32r identity: memset must run on an f32 view (Memset of f32r
        # fails the ISA check) while the affine_select writes the f32r view
        # so downstream f32r matmuls see properly rounded inputs.
        nc.gpsimd.memset(ident[:].bitcast(F32), 0.0)
        nc.gpsimd.affine_select(
            out=ident[:],
            in_=ident[:],
            compare_op=mybir.AluOpType.not_equal,
            fill=1.0,
            base=0,
            pattern=[[-1, 128]],
            channel_multiplier=1,
        )
        ones_stage = const.tile([128, 64], F32)

        # ---- loads. HWDGE costs ~625ns of descriptor generation per
        # dma_start regardless of size, so tiles are fetched in k-PAIRS for
        # the latency-critical prologue (projection k-steps consume pairs)
        # and in bigger merged transfers for everything later.
        xT_p = xT.rearrange("(t p) s -> p t s", p=128)
        wpk_p = wpk.rearrange("(t p) m -> p t m", p=128)
        for t0 in range(0, NKD, 2):
            nc.sync.dma_start(
                out=wpk_sb[:, t0 : t0 + 2, 0:384], in_=wpk_p[:, t0 : t0 + 2, 0:384]
            )
            nc.sync.dma_start(
                out=xT_sb[:, t0 : t0 + 2, 0:QT], in_=xT_p[:, t0 : t0 + 2, 0:QT]
            )
        nc.sync.dma_start(out=bqk_sb[:], in_=bqk[:, :])
        nc.sync.dma_start(out=wpk_sb[:, :, 384:WPK], in_=wpk_p[:, :, 384:WPK])
        nc.sync.dma_start(out=ones_stage[:], in_=vones[:, :])
        nc.sync.dma_start(out=ones64[:], in_=vones[0:1, 0:64].bitcast(F32R))
        nc.vector.tensor_copy(
            out=v_aug[:, :, :, DH],
            in_=ones_stage[:, 0 : HPC * NKT]
            .rearrange("p (h t) -> p h t", h=HPC)
            .bitcast(F32R),
        )
        for t0 in range(0, NKD, 3):
            nc.sync.dma_start(
                out=xT_sb[:, t0 : t0 + 3, QT : 2 * QT],
                in_=xT_p[:, t0 : t0 + 3, QT : 2 * QT],
            )
        nc.sync.dma_start(out=wo_a[:], in_=woT[0:128, :].bitcast(F32R))
        nc.sync.dma_start(out=wo_b[:], in_=woT[128:GD, :].bitcast(F32R))
        for t0 in range(0, NKD, 3):
            nc.sync.dma_start(
                out=xT_sb[:, t0 : t0 + 3, 2 * QT : S],
                in_=xT_p[:, t0 : t0 + 3, 2 * QT : S],
            )

        # packed projection m-groups: (col0, rows, evict spec)
        # evict spec: list of (psum row range, dst ap fn, bias col or None)
        def ev_q(col):
            return lambda n, r0, r1: qT_sb[r0:r1, col, n * QT : (n + 1) * QT]

        def ev_k(col):
            return lambda n, r0, r1: kT_sb[r0:r1, col, n * QT : (n + 1) * QT]

        def ev_v(col):
            return lambda n, r0, r1: vvT[r0:r1, col, n * QT : (n + 1) * QT]

        mgroups = [
            (0, 128, [((0, 128), ev_q(0), 0)]),
            (128, 128, [((0, 64), ev_q(1), 1), ((64, 128), ev_v(0), None)]),
            (256, 128, [((0, 128), ev_k(0), 2)]),
            (384, 128, [((0, 64), ev_k(1), 3), ((64, 128), ev_v(1), None)]),
            (512, 64, [((0, 64), ev_v(0), None)]),
        ]
        # v pieces: v_h0 -> vvT[64:128, 0], v_h1 -> vvT[64:128, 1],
        # v_h2 -> vvT[0:64, 0] (from the m4 group, psum rows 0:64)

        proj_psums = {}

        def proj_unit(mi, n, kpair, evict_act=False):
            """Two K-step matmuls of group (mi, n); evictions after the last."""
            c0, msz, evicts = mgroups[mi]
            key = (mi, n)
            if key not in proj_psums:
                proj_psums[key] = ps.tile([128, QT], F32, tag="ps", name="projp")
            p = proj_psums[key]
            for k in (2 * kpair, 2 * kpair + 1):
                nc.tensor.matmul(
                    p[:msz, :],
                    lhsT=wpk_sb[:, k, c0 : c0 + msz],
                    rhs=xT_sb[:, k, n * QT : (n + 1) * QT],
                    start=(k == 0),
                    stop=(k == NKD - 1),
                )
            if kpair == 2:
                del proj_psums[key]
                for (r0, r1), dst, bcol in evicts:
                    if mi == 4:
                        dst_ap = dst(n, 0, 64)  # v_h2 rows live at psum 0:64
                    else:
                        dst_ap = dst(n, r0, r1)
                    if bcol is None:
                        nc.vector.tensor_copy(out=dst_ap, in_=p[r0:r1, :])
                        continue
                    # NOTE: GPSIMD/Pool cannot read PSUM, so eviction
                    # offload is limited to ACT (Identity = copy + bias).
                    if (
                        (EV_SPLIT == "act0" and mi == 0)
                        or (EV_SPLIT == "act02" and mi in (0, 2))
                        or (EV_SPLIT == "actn1" and evict_act and mi in (0, 2))
                    ):
                        nc.scalar.activation(
                            out=dst_ap,
                            in_=p[r0:r1, :],
                            func=AF.Identity,
                            bias=bqk_sb[r0:r1, bcol : bcol + 1],
                        )
                    else:
                        nc.vector.tensor_scalar_add(
                            out=dst_ap,
                            in0=p[r0:r1, :],
                            scalar1=bqk_sb[r0:r1, bcol : bcol + 1],
                        )

        def transpose_unit(t, piece):
            """piece 0/1/2 = head 0/1/2; v_h0/v_h1 at vvT[64:128,0/1], v_h2 at vvT[0:64,0]."""
            if piece == 2:
                src = vvT[0:64, 0, t * 128 : (t + 1) * 128]
                idn = ident[0:64, 0:64]
            else:
                src = vvT[64:128, piece, t * 128 : (t + 1) * 128]
                idn = ident[64:128, 64:128]
            pt = ps.tile([128, QT], F32R, tag="ps")
            nc.tensor.transpose(pt[:, 0:64], src, idn)
            nc.vector.tensor_copy(v_aug[:, piece, t, 0:64], pt[:, 0:64])

        out_pair = out_p.rearrange("(tp a p) d -> tp p a d", a=2, p=128)
        out_sing = out_p.rearrange("(t p) d -> t p d", p=128)
        o_pairs = {}

        def o_proj_unit(t, n2, solo_dma=False):
            key = t // 2
            po = ps.tile([128, QT], F32, tag="ps")
            nc.tensor.matmul(
                po[:, 0:384],
                lhsT=zT01[:, t * 128 : (t + 1) * 128],
                rhs=wo_a[:, n2 * 384 : (n2 + 1) * 384],
                start=True,
                stop=False,
            )
            nc.tensor.matmul(
                po[:, 0:384],
                lhsT=zT2[:, t * 128 : (t + 1) * 128],
                rhs=wo_b[:, n2 * 384 : (n2 + 1) * 384],
                start=False,
                stop=True,
            )
            if solo_dma:
                # end of kernel: copies alternate ACT/DVE so the two halves
                # stage in parallel, then one per-tile store fires.
                ob = o_pairs.setdefault(
                    key, expp.tile([128, 2, D], OUT_DT, tag="osb", name="osb", bufs=2)
                )
                dst = ob[:, t % 2, n2 * 384 : (n2 + 1) * 384]
                if FCOPY == "act" or n2 == 0:
                    nc.scalar.activation(out=dst, in_=po[:, 0:384], func=AF.Copy)
                else:
                    nc.vector.tensor_copy(out=dst, in_=po[:, 0:384])
                if n2 == 1:
                    nc.sync.dma_start(out=out_sing[t], in_=ob[:, t % 2, :])
                    if t % 2 == 1:
                        del o_pairs[key]
                return
            if key not in o_pairs:
                o_pairs[key] = expp.tile([128, 2, D], OUT_DT, tag="osb", name="osb", bufs=2)
            ob = o_pairs[key]
            if t >= 12 and (t + n2) % 2 == 0:
                nc.scalar.activation(
                    out=ob[:, t % 2, n2 * 384 : (n2 + 1) * 384],
                    in_=po[:, 0:384],
                    func=AF.Copy,
                )
            else:
                nc.vector.tensor_copy(
                    out=ob[:, t % 2, n2 * 384 : (n2 + 1) * 384], in_=po[:, 0:384]
                )
            if t % 2 == 1 and n2 == 1:
                del o_pairs[key]
                nc.sync.dma_start(out=out_pair[key], in_=ob[:, :, :])

        # background work queue of (key, fn), drained between attention
        # iterations. Queue order is topological (a group's transposes come
        # after its evictions), so force-draining "through the last needed
        # unit" preserves all producer->consumer program ordering.
        work = deque()

        def q_proj(n, mis=range(5)):
            ea = n >= 1 and EV_SPLIT == "actn1"
            for mi in mis:
                for kpair in range(3):
                    work.append(
                        (
                            ("proj", n, mi),
                            lambda mi=mi, n=n, kp=kpair, ea=ea: proj_unit(
                                mi, n, kp, evict_act=ea
                            ),
                        )
                    )

        trq = deque()  # transpose units, pulled only by ensure_tr (PV time)

        def q_tr(ts, pieces=range(HPC)):
            for t in ts:
                for piece in pieces:
                    trq.append(
                        ((t, piece), lambda t=t, p=piece: transpose_unit(t, p))
                    )

        def drain(k=1):
            for _ in range(k):
                if work:
                    work.popleft()[1]()

        def drain_all():
            while work:
                work.popleft()[1]()
            while trq:
                trq.popleft()[1]()

        PROJ_GROUPS_FOR_HEAD = {0: (0, 1, 2), 1: (0, 2, 3), 2: (1, 3, 4)}

        def _drain_through(needed):
            last = -1
            for i, (key, _) in enumerate(work):
                if key in needed:
                    last = i
            for _ in range(last + 1):
                work.popleft()[1]()

        def force_drain_for(h, qt):
            """Emit queued units the SCORES of attention(h, qt) depend on.
            V transposes are pulled lazily by the PV closures instead."""
            needed = set()
            for n in range(qt + 1):
                for mi in PROJ_GROUPS_FOR_HEAD[h]:
                    needed.add(("proj", n, mi))
            _drain_through(needed)

        def ensure_tr(t, piece):
            last = -1
            for i, (key, _) in enumerate(trq):
                if key == (t, piece):
                    last = i
            for _ in range(last + 1):
                trq.popleft()[1]()

        def qh(h):
            m, off = divmod(h * 64, 128)
            return qT_sb[off : off + 64, m, :]

        def kh(h):
            m, off = divmod(h * 64, 128)
            return kT_sb[off : off + 64, m, :]

        zdst = [zT01[0:64, :], zT01[64:128, :], zT2[0:64, :]]

        # PV matmuls are pipelined ~4 iterations behind their exp across
        # block boundaries, so the in-order PE FIFO never waits on the
        # exp/mask chain, not even at the end of a block.
        pvq = deque()  # (block_serial, pv_closure)
        blk_serial = [0]

        def pv_drain(depth):
            while len(pvq) > depth:
                pvq.popleft()[1]()

        def pv_flush(upto_serial):
            while pvq and pvq[0][0] <= upto_serial:
                pvq.popleft()[1]()

        def lo_of(rr):
            # query-column start of the computed region for a key tile with
            # diagonal offset rr; rr=3 is held at 256 so the fp32r matmuls
            # stay >=256 wide (narrower pays 4x in rate).
            if rr <= 0:
                return 0
            return 128 * rr if rr < 3 else 256

        def attention(h, qt, per_kt, pvdepth=None):
            """scores^T -> exp -> causal mask (narrow zone) -> PV into zp."""
            if pvdepth is None:
                pvdepth = PVQ
            zp = psz.tile([DH + 1, QT], F32)
            nkt = 4 * qt + 4
            blk = blk_serial[0]
            blk_serial[0] += 1

            def pv(kt, es, lo):
                ensure_tr(kt, h)
                nc.tensor.matmul(
                    zp[:, lo:QT],
                    lhsT=v_aug[:, h, kt, :],
                    rhs=es[:, lo:QT],
                    start=(kt == 0),
                    stop=(kt == nkt - 1),
                )

            for kt in range(nkt):
                rr = kt - 4 * qt
                lo = lo_of(rr)
                sp = ps.tile([128, QT], F32, tag="ps")
                nc.tensor.matmul(
                    sp[:, lo:QT],
                    lhsT=kh(h)[:, kt * 128 : (kt + 1) * 128],
                    rhs=qh(h)[:, qt * QT + lo : (qt + 1) * QT],
                    start=True,
                    stop=True,
                )
                es = expp.tile([128, QT], F32R, tag="expp")
                nc.scalar.activation(out=es[:, lo:QT], in_=sp[:, lo:QT], func=AF.Exp)
                if rr >= 0:  # diagonal: zero where key > query
                    z0 = 128 * rr if rr < 3 else 256
                    zw = 128 if rr < 3 else 256
                    nc.gpsimd.affine_select(
                        out=es[:, z0 : z0 + zw],
                        in_=es[:, z0 : z0 + zw],
                        compare_op=mybir.AluOpType.is_ge,
                        fill=0.0,
                        base=z0 - 128 * rr,
                        channel_multiplier=-1,
                        pattern=[[1, zw]],
                    )
                pvq.append((blk, lambda kt=kt, es=es, lo=lo: pv(kt, es, lo)))
                if per_kt in (2, 3, 4):
                    drain(per_kt)
                elif per_kt == 9:
                    drain(1)
                elif kt % 2 == 0:
                    drain(1)
                pv_drain(pvdepth)
            return zp, blk

        def normalize(zp, h, qt, cols=slice(0, QT)):
            rec = small.tile([1, QT], F32R, tag="rec")
            with nc.allow_low_precision(reason="f32r is fp32-precision"):
                nc.vector.reciprocal(rec[:, cols], zp[DH : DH + 1, cols])
            bc = ps.tile([128, QT], F32, tag="ps")
            nc.tensor.matmul(
                bc[0:64, cols], lhsT=ones64[:], rhs=rec[:, cols], start=True, stop=True
            )
            bc_sb = small.tile([64, QT], F32, tag="bcsb")
            if qt == NQT - 1:
                nc.vector.tensor_copy(out=bc_sb[:, cols], in_=bc[0:64, cols])
            else:
                nc.scalar.activation(out=bc_sb[:, cols], in_=bc[0:64, cols], func=AF.Copy)
            nc.vector.tensor_mul(
                zdst[h][:, qt * QT : (qt + 1) * QT][:, cols],
                zp[0:DH, cols],
                bc_sb[:, cols],
            )

        # ---- schedule ----
        # prologue: only what attention(h0, qt0) needs; the rest queues up.
        for mi in (0, 2, 1):
            for kpair in range(3):
                proj_unit(mi, 0, kpair)
        q_tr(range(4), pieces=(0,))
        q_proj(0, mis=(3,))
        q_tr(range(4), pieces=(1,))
        q_proj(0, mis=(4,))
        q_tr(range(4), pieces=(2,))
        for n in range(1, NQT):
            q_proj(n, mis=(0, 2, 1))
            q_tr(range(4 * n, 4 * n + 4), pieces=(0,))
            q_proj(n, mis=(3,))
            q_tr(range(4 * n, 4 * n + 4), pieces=(1,))
            q_proj(n, mis=(4,))
            q_tr(range(4 * n, 4 * n + 4), pieces=(2,))

        pending = None
        for qt in range(NQT):
            per_kt = [PACE0, 1, 1, PACE3][qt]
            for h in range(HPC):
                if pending is not None and NORM_EARLY:
                    # normalize the pending block BEFORE the next block's
                    # eviction burst so its reciprocal isn't queued behind
                    # them on DVE (the PE-side broadcast waits on it)
                    pv_flush(pending[3])
                    normalize(*pending[:3])
                    ph, pqt = pending[1], pending[2]
                    if ph == HPC - 1:
                        for t in range(4 * pqt, 4 * pqt + 4):
                            for n2 in range(2):
                                work.append(
                                    (("o", pqt), lambda t=t, n2=n2: o_proj_unit(t, n2))
                                )
                    pending = None
                force_drain_for(h, qt)
                zp, blk = attention(
                    h, qt, per_kt,
                    pvdepth=LPVQ if (qt == NQT - 1 and h == HPC - 1) else PVQ,
                )
                if pending is not None:
                    pv_flush(pending[3])  # pending block's PV accumulation done
                    normalize(*pending[:3])
                    ph, pqt = pending[1], pending[2]
                    if ph == HPC - 1:  # whole q-tile normalized -> O-proj ready
                        for t in range(4 * pqt, 4 * pqt + 4):
                            for n2 in range(2):
                                work.append(
                                    (("o", pqt), lambda t=t, n2=n2: o_proj_unit(t, n2))
                                )
                pending = (zp, h, qt, blk)
        # final block: normalize in column halves so the last O-proj pairs
        # start while the second half's recip/broadcast chain is still running
        pv_flush(pending[3])
        drain_all()
        if EPI == "quarter":
            for quarter in range(4):
                normalize(*pending[:3], cols=slice(quarter * 128, (quarter + 1) * 128))
                for n2 in range(2):
                    o_proj_unit(12 + quarter, n2, solo_dma=True)
        else:
            for half in range(2):
                normalize(*pending[:3], cols=slice(half * 256, (half + 1) * 256))
                for t in (12 + 2 * half, 13 + 2 * half):
                    for n2 in range(2):
                        o_proj_unit(t, n2, solo_dma=True)
    nc.finalize()
    return nc


_NC_CACHE = {}


def make_in_maps(x, W_qkv, b_qkv, W_o):
    in_maps = []
    for c in range(8):
        b, g = divmod(c, 4)
        hs = [HPC * g + i for i in range(HPC)]
        qr = [np.arange(64 * h, 64 * h + 64) for h in hs]
        w_q = [W_qkv[i] * 0.125 for i in qr]
        w_k = [W_qkv[768 + i] for i in qr]
        w_v = [W_qkv[1536 + i] for i in qr]
        b_q = [b_qkv[i] * 0.125 for i in qr]
        b_k = [b_qkv[768 + i] for i in qr]
        # packed rows: m0=[q0 q1] m1=[q2 v0] m2=[k0 k1] m3=[k2 v1] m4=[v2]
        wpk = np.concatenate(
            [w_q[0], w_q[1], w_q[2], w_v[0], w_k[0], w_k[1], w_k[2], w_v[1], w_v[2]],
            axis=0,
        )
        bqk_col = np.zeros((128, 4), np.float32)
        bqk_col[:, 0] = np.concatenate([b_q[0], b_q[1]])
        bqk_col[0:64, 1] = b_q[2]
        bqk_col[:, 2] = np.concatenate([b_k[0], b_k[1]])
        bqk_col[0:64, 3] = b_k[2]
        in_dt = ml_dtypes.bfloat16 if os.environ.get("K_BF16", "1") == "1" else np.float32
        in_maps.append(
            {
                "xT": np.ascontiguousarray(x[b].T).astype(in_dt),
                "wpk": np.ascontiguousarray(wpk.T).astype(in_dt),
                "woT": np.ascontiguousarray(W_o[:, GD * g : GD * (g + 1)].T),
                "bqk": bqk_col,
                "vones": np.ones((128, 64), np.float32),
            }
        )
    return in_maps


def make_in_maps_for_test(inputs):
    return make_in_maps(
        np.asarray(inputs["x"], np.float32),
        np.asarray(inputs["W_qkv"], np.float32),
        np.asarray(inputs["b_qkv"], np.float32),
        np.asarray(inputs["W_o"], np.float32),
    )


def kernel(x, W_qkv, b_qkv, W_o, b_o):
    x = np.asarray(x, np.float32)
    W_qkv = np.asarray(W_qkv, np.float32)
    b_qkv = np.asarray(b_qkv, np.float32)
    W_o = np.asarray(W_o, np.float32)
    b_o = np.asarray(b_o, np.float32)

    if "nc" not in _NC_CACHE:
        _NC_CACHE["nc"] = build_bass()
    nc = _NC_CACHE["nc"]

    in_maps = make_in_maps(x, W_qkv, b_qkv, W_o)

    res = run_bass_kernel_spmd(
        nc,
        in_maps,
        list(range(8)),
        trace=bool(int(os.environ.get("KERNEL_TRACE", "0"))),
    )
    _NC_CACHE["last_results"] = res

    out = np.zeros((B, S, D), np.float32)
    for c in range(8):
        out[c // 4] += np.asarray(res.results[c]["out_p"], np.float32)
    out += b_qkv[1536:] @ W_o.T + b_o
    return out


# revision 32
# speedup vs baseline: 1.0868x; 1.0114x over previous
"""Causal multi-head attention block (B=2, S=2048, D=768, H=12) on 8 trn2 cores.

Sharding: core c -> batch b = c//4 (data parallel), head group g = c%4
(tensor parallel, 3 heads per group). Each core computes its group's QKV
projection, causal attention, and a partial O-projection over its 192
z-columns. Host sums the 4 partials per batch and adds the biases that
commute through the math (v-bias and b_o).

On-core layout (everything "transposed", d on partitions, seq on free):
  xT   [768, 2048]   q/kT  [64*, 2048]      scores^T [keys, q]
so the softmax denominator comes free from a ones-column appended to V in
the PV matmul, and no on-chip transposes of activations are needed except
V (built via PE transpose from V^T).

The QKV projection uses a host-repacked weight matrix so every 128-wide
M-group is fully used:
  m0=[q_h0 q_h1] m1=[q_h2 v_h0] m2=[k_h0 k_h1] m3=[k_h2 v_h1] m4=[v_h2]
(q rows pre-scaled by 1/8; v bias folded into the host-side epilogue).

Matmul operands are float32r (full-rate fp32 on the PE). Scheduling
interleaves projection/transpose/O-proj work into the attention loop so
the scalar engine (exp) is never starved by a long PE FIFO stretch.

Cost-structure details:
  - fp32r matmuls under 256 output columns run at 1/4 rate, so the rr=3
    diagonal block is computed 256 wide (the causal mask zeroes the
    overhang) instead of 128 wide.
  - causal masks only touch the 128/256-wide zone that can violate
    causality instead of the whole remaining tile.
  - V^T and the identity live in f32r so the V transposes run at 1.5
    cycles/row instead of fp32's 2.0.
  - dummy PE transposes during the DMA prologue keep the tensor engine
    busy so the p-state ramp finishes before real matmuls arrive.
  - the last four token tiles store per 384-column half as soon as each
    O-proj eviction lands, shortening the end-of-kernel DMA tail.
"""

import os
from collections import deque
from contextlib import ExitStack

import numpy as np
import ml_dtypes

import concourse.tile as tile
from concourse import bacc, mybir
from concourse.bass_utils import run_bass_kernel_spmd

F32 = mybir.dt.float32
F32R = mybir.dt.float32r
AF = mybir.ActivationFunctionType

B, S, D = 2, 2048, 768
NH, DH = 12, 64
HPC = 3            # heads per core
GD = HPC * DH      # 192 z-cols per core
KT, QT = 128, 512  # key tile (partitions), q tile (psum free)
NKT, NQT = S // KT, S // QT   # 16, 4
NTOK = S // 128    # 16 token tiles
NKD = D // 128     # 6 contraction tiles for the projections
WPK = 2 * GD + GD  # 576 packed projection rows

N_WARMUP = int(os.environ.get("K_WARMUP", "16"))   # dummy PE transposes in prologue
PVQ = int(os.environ.get("K_PVQ", "7"))            # PV queue lag depth
LPVQ = int(os.environ.get("K_LPVQ", "7"))          # PV lag depth for the final block
FCOPY = os.environ.get("K_FCOPY", "mix")           # act|mix: final 4-tile eviction engines
NORM_EARLY = os.environ.get("K_NORME", "0") == "1" # normalize before next force_drain
PACE3 = int(os.environ.get("K_PACE3", "9"))        # qt3 drain mode
BFPV = os.environ.get("K_BFPV", "1") == "1"        # bf16 es/V path (PV + transposes)
BF16_IN = os.environ.get("K_BF16", "1") == "1"     # load x / W_qkv in bf16
BF16_OUT = os.environ.get("K_BF16O", "1") == "1"   # store O-proj partials in bf16
EV_SPLIT = os.environ.get("K_EVSPLIT", "none")   # none|pool2|pool23: proj evictions offload
PACE0 = int(os.environ.get("K_PACE0", "2"))        # qt0 drain units per kt
EPI = os.environ.get("K_EPI", "half")           # half|quarter: final normalize granularity


def build_bass():
    nc = bacc.Bacc(None)
    in_dt_d = mybir.dt.bfloat16 if BF16_IN else F32
    xT = nc.dram_tensor("xT", [D, S], in_dt_d, kind="ExternalInput")
    wpk = nc.dram_tensor("wpk", [D, WPK], in_dt_d, kind="ExternalInput")
    woT = nc.dram_tensor("woT", [GD, D], F32, kind="ExternalInput")
    bqk = nc.dram_tensor("bqk", [128, 4], F32, kind="ExternalInput")
    vones = nc.dram_tensor("vones", [128, 64], F32, kind="ExternalInput")
    out_dt_d = mybir.dt.bfloat16 if BF16_OUT else F32
    out_p = nc.dram_tensor("out_p", [S, D], out_dt_d, kind="ExternalOutput")

    with tile.TileContext(nc) as tc, ExitStack() as ctx:
        const = ctx.enter_context(tc.tile_pool(name="const", bufs=1))
        ps = ctx.enter_context(tc.tile_pool(name="ps", bufs=6, space="PSUM"))
        psz = ctx.enter_context(tc.tile_pool(name="psz", bufs=2, space="PSUM"))
        expp = ctx.enter_context(tc.tile_pool(name="expp", bufs=9))
        small = ctx.enter_context(tc.tile_pool(name="small", bufs=4))

        IN_DT = mybir.dt.bfloat16 if BF16_IN else F32R
        OUT_DT = mybir.dt.bfloat16 if BF16_OUT else F32
        xT_sb = const.tile([128, NKD, S], IN_DT)
        wpk_sb = const.tile([128, NKD, WPK], IN_DT)
        wo_a = const.tile([128, D], F32R)
        wo_b = const.tile([64, D], F32R)
        bqk_sb = const.tile([128, 4], F32)
        qT_sb = const.tile([128, 2, S], F32R)
        kT_sb = const.tile([128, 2, S], F32R)
        PV_DT = mybir.dt.bfloat16 if BFPV else F32R
        vvT = const.tile([128, 2, S], PV_DT)
        v_aug = const.tile([128, HPC, NKT, DH + 1], PV_DT)
        zT01 = const.tile([128, S], F32R)
        zT2 = const.tile([64, S], F32R)
        ident = const.tile([128, 128], F32R)
        ones64 = const.tile([1, 64], F32R)

        # warmup: keep the PE busy while the first DMAs land so the p-state
        # ramp is done before real matmuls issue. The transposes read the
        # not-yet-written identity tile; the values are irrelevant (nothing
        # reads `warm`) and the WAR ordering only delays make_identity to
        # ~1.5us, well before the first real V transpose needs it.
        warm = ps.tile([128, QT], F32R, tag="ps", name="warm")
        for _ in range(N_WARMUP):
            nc.tensor.transpose(warm[:, 0:128], ident[:, :], ident[:, :])

        # f32r identity: memset must run on an f32 view (Memset of f32r
        # fails the ISA check) while the affine_select writes the f32r view
        # so downstream f32r matmuls see properly rounded inputs.
        nc.gpsimd.memset(ident[:].bitcast(F32), 0.0)
        nc.gpsimd.affine_select(
            out=ident[:],
            in_=ident[:],
            compare_op=mybir.AluOpType.not_equal,
            fill=1.0,
            base=0,
            pattern=[[-1, 128]],
            channel_multiplier=1,
        )
        ident_pv = const.tile([128, 128], PV_DT)
        nc.vector.tensor_copy(out=ident_pv[:], in_=ident[:])
        ones_stage = const.tile([128, 64], F32)

        # ---- loads. HWDGE costs ~625ns of descriptor generation per
        # dma_start regardless of size, so tiles are fetched in k-PAIRS for
        # the latency-critical prologue (projection k-steps consume pairs)
        # and in bigger merged transfers for everything later.
        xT_p = xT.rearrange("(t p) s -> p t s", p=128)
        wpk_p = wpk.rearrange("(t p) m -> p t m", p=128)
        for t0 in range(0, NKD, 2):
            nc.sync.dma_start(
                out=wpk_sb[:, t0 : t0 + 2, 0:384], in_=wpk_p[:, t0 : t0 + 2, 0:384]
            )
            nc.sync.dma_start(
                out=xT_sb[:, t0 : t0 + 2, 0:QT], in_=xT_p[:, t0 : t0 + 2, 0:QT]
            )
        nc.sync.dma_start(out=bqk_sb[:], in_=bqk[:, :])
        nc.sync.dma_start(out=wpk_sb[:, :, 384:WPK], in_=wpk_p[:, :, 384:WPK])
        nc.sync.dma_start(out=ones_stage[:], in_=vones[:, :])
        nc.sync.dma_start(out=ones64[:], in_=vones[0:1, 0:64].bitcast(F32R))
        nc.vector.tensor_copy(
            out=v_aug[:, :, :, DH],
            in_=ones_stage[:, 0 : HPC * NKT].rearrange("p (h t) -> p h t", h=HPC),
        )
        for t0 in range(0, NKD, 3):
            nc.sync.dma_start(
                out=xT_sb[:, t0 : t0 + 3, QT : 2 * QT],
                in_=xT_p[:, t0 : t0 + 3, QT : 2 * QT],
            )
        nc.sync.dma_start(out=wo_a[:], in_=woT[0:128, :].bitcast(F32R))
        nc.sync.dma_start(out=wo_b[:], in_=woT[128:GD, :].bitcast(F32R))
        for t0 in range(0, NKD, 3):
            nc.sync.dma_start(
                out=xT_sb[:, t0 : t0 + 3, 2 * QT : S],
                in_=xT_p[:, t0 : t0 + 3, 2 * QT : S],
            )

        # packed projection m-groups: (col0, rows, evict spec)
        # evict spec: list of (psum row range, dst ap fn, bias col or None)
        def ev_q(col):
            return lambda n, r0, r1: qT_sb[r0:r1, col, n * QT : (n + 1) * QT]

        def ev_k(col):
            return lambda n, r0, r1: kT_sb[r0:r1, col, n * QT : (n + 1) * QT]

        def ev_v(col):
            return lambda n, r0, r1: vvT[r0:r1, col, n * QT : (n + 1) * QT]

        mgroups = [
            (0, 128, [((0, 128), ev_q(0), 0)]),
            (128, 128, [((0, 64), ev_q(1), 1), ((64, 128), ev_v(0), None)]),
            (256, 128, [((0, 128), ev_k(0), 2)]),
            (384, 128, [((0, 64), ev_k(1), 3), ((64, 128), ev_v(1), None)]),
            (512, 64, [((0, 64), ev_v(0), None)]),
        ]
        # v pieces: v_h0 -> vvT[64:128, 0], v_h1 -> vvT[64:128, 1],
        # v_h2 -> vvT[0:64, 0] (from the m4 group, psum rows 0:64)

        proj_psums = {}

        def proj_unit(mi, n, kpair, evict_act=False):
            """Two K-step matmuls of group (mi, n); evictions after the last."""
            c0, msz, evicts = mgroups[mi]
            key = (mi, n)
            if key not in proj_psums:
                proj_psums[key] = ps.tile([128, QT], F32, tag="ps", name="projp")
            p = proj_psums[key]
            for k in (2 * kpair, 2 * kpair + 1):
                nc.tensor.matmul(
                    p[:msz, :],
                    lhsT=wpk_sb[:, k, c0 : c0 + msz],
                    rhs=xT_sb[:, k, n * QT : (n + 1) * QT],
                    start=(k == 0),
                    stop=(k == NKD - 1),
                )
            if kpair == 2:
                del proj_psums[key]
                for (r0, r1), dst, bcol in evicts:
                    if mi == 4:
                        dst_ap = dst(n, 0, 64)  # v_h2 rows live at psum 0:64
                    else:
                        dst_ap = dst(n, r0, r1)
                    if bcol is None:
                        nc.vector.tensor_copy(out=dst_ap, in_=p[r0:r1, :])
                        continue
                    # NOTE: GPSIMD/Pool cannot read PSUM, so eviction
                    # offload is limited to ACT (Identity = copy + bias).
                    if (
                        (EV_SPLIT == "act0" and mi == 0)
                        or (EV_SPLIT == "act02" and mi in (0, 2))
                        or (EV_SPLIT == "actn1" and evict_act and mi in (0, 2))
                    ):
                        nc.scalar.activation(
                            out=dst_ap,
                            in_=p[r0:r1, :],
                            func=AF.Identity,
                            bias=bqk_sb[r0:r1, bcol : bcol + 1],
                        )
                    else:
                        nc.vector.tensor_scalar_add(
                            out=dst_ap,
                            in0=p[r0:r1, :],
                            scalar1=bqk_sb[r0:r1, bcol : bcol + 1],
                        )

        def transpose_unit(t, piece):
            """piece 0/1/2 = head 0/1/2; v_h0/v_h1 at vvT[64:128,0/1], v_h2 at vvT[0:64,0]."""
            if piece == 2:
                src = vvT[0:64, 0, t * 128 : (t + 1) * 128]
                idn = ident_pv[0:64, 0:64]
            else:
                src = vvT[64:128, piece, t * 128 : (t + 1) * 128]
                idn = ident_pv[64:128, 64:128]
            pt = ps.tile([128, QT], PV_DT, tag="ps")
            nc.tensor.transpose(pt[:, 0:64], src, idn)
            nc.vector.tensor_copy(v_aug[:, piece, t, 0:64], pt[:, 0:64])

        out_pair = out_p.rearrange("(tp a p) d -> tp p a d", a=2, p=128)
        out_sing = out_p.rearrange("(t p) d -> t p d", p=128)
        o_pairs = {}

        def o_proj_unit(t, n2, solo_dma=False):
            key = t // 2
            po = ps.tile([128, QT], F32, tag="ps")
            nc.tensor.matmul(
                po[:, 0:384],
                lhsT=zT01[:, t * 128 : (t + 1) * 128],
                rhs=wo_a[:, n2 * 384 : (n2 + 1) * 384],
                start=True,
                stop=False,
            )
            nc.tensor.matmul(
                po[:, 0:384],
                lhsT=zT2[:, t * 128 : (t + 1) * 128],
                rhs=wo_b[:, n2 * 384 : (n2 + 1) * 384],
                start=False,
                stop=True,
            )
            if solo_dma:
                # end of kernel: copies alternate ACT/DVE so the two halves
                # stage in parallel, then one per-tile store fires.
                ob = o_pairs.setdefault(
                    key, expp.tile([128, 2, D], OUT_DT, tag="osb", name="osb", bufs=2)
                )
                dst = ob[:, t % 2, n2 * 384 : (n2 + 1) * 384]
                if FCOPY == "act" or n2 == 0:
                    nc.scalar.activation(out=dst, in_=po[:, 0:384], func=AF.Copy)
                else:
                    nc.vector.tensor_copy(out=dst, in_=po[:, 0:384])
                if n2 == 1:
                    nc.sync.dma_start(out=out_sing[t], in_=ob[:, t % 2, :])
                    if t % 2 == 1:
                        del o_pairs[key]
                return
            if key not in o_pairs:
                o_pairs[key] = expp.tile([128, 2, D], OUT_DT, tag="osb", name="osb", bufs=2)
            ob = o_pairs[key]
            if t >= 12 and (t + n2) % 2 == 0:
                nc.scalar.activation(
                    out=ob[:, t % 2, n2 * 384 : (n2 + 1) * 384],
                    in_=po[:, 0:384],
                    func=AF.Copy,
                )
            else:
                nc.vector.tensor_copy(
                    out=ob[:, t % 2, n2 * 384 : (n2 + 1) * 384], in_=po[:, 0:384]
                )
            if t % 2 == 1 and n2 == 1:
                del o_pairs[key]
                nc.sync.dma_start(out=out_pair[key], in_=ob[:, :, :])

        # background work queue of (key, fn), drained between attention
        # iterations. Queue order is topological (a group's transposes come
        # after its evictions), so force-draining "through the last needed
        # unit" preserves all producer->consumer program ordering.
        work = deque()

        def q_proj(n, mis=range(5)):
            ea = n >= 1 and EV_SPLIT == "actn1"
            for mi in mis:
                for kpair in range(3):
                    work.append(
                        (
                            ("proj", n, mi),
                            lambda mi=mi, n=n, kp=kpair, ea=ea: proj_unit(
                                mi, n, kp, evict_act=ea
                            ),
                        )
                    )

        trq = deque()  # transpose units, pulled only by ensure_tr (PV time)

        def q_tr(ts, pieces=range(HPC)):
            for t in ts:
                for piece in pieces:
                    trq.append(
                        ((t, piece), lambda t=t, p=piece: transpose_unit(t, p))
                    )

        def drain(k=1):
            for _ in range(k):
                if work:
                    work.popleft()[1]()

        def drain_all():
            while work:
                work.popleft()[1]()
            while trq:
                trq.popleft()[1]()

        PROJ_GROUPS_FOR_HEAD = {0: (0, 1, 2), 1: (0, 2, 3), 2: (1, 3, 4)}

        def _drain_through(needed):
            last = -1
            for i, (key, _) in enumerate(work):
                if key in needed:
                    last = i
            for _ in range(last + 1):
                work.popleft()[1]()

        def force_drain_for(h, qt):
            """Emit queued units the SCORES of attention(h, qt) depend on.
            V transposes are pulled lazily by the PV closures instead."""
            needed = set()
            for n in range(qt + 1):
                for mi in PROJ_GROUPS_FOR_HEAD[h]:
                    needed.add(("proj", n, mi))
            _drain_through(needed)

        def ensure_tr(t, piece):
            last = -1
            for i, (key, _) in enumerate(trq):
                if key == (t, piece):
                    last = i
            for _ in range(last + 1):
                trq.popleft()[1]()

        def qh(h):
            m, off = divmod(h * 64, 128)
            return qT_sb[off : off + 64, m, :]

        def kh(h):
            m, off = divmod(h * 64, 128)
            return kT_sb[off : off + 64, m, :]

        zdst = [zT01[0:64, :], zT01[64:128, :], zT2[0:64, :]]

        # PV matmuls are pipelined ~4 iterations behind their exp across
        # block boundaries, so the in-order PE FIFO never waits on the
        # exp/mask chain, not even at the end of a block.
        pvq = deque()  # (block_serial, pv_closure)
        blk_serial = [0]

        def pv_drain(depth):
            while len(pvq) > depth:
                pvq.popleft()[1]()

        def pv_flush(upto_serial):
            while pvq and pvq[0][0] <= upto_serial:
                pvq.popleft()[1]()

        def lo_of(rr):
            # query-column start of the computed region for a key tile with
            # diagonal offset rr; rr=3 is held at 256 so the fp32r SCORES
            # matmul stays >=256 wide (narrower pays 4x in rate).
            if rr <= 0:
                return 0
            return 128 * rr if rr < 3 else 256

        def pv_lo_of(rr):
            # bf16 PV has no narrow-width penalty, so the rr=3 PV reads just
            # the 128 columns that survive the causal mask
            if BFPV and rr == 3:
                return 384
            return lo_of(rr)

        def attention(h, qt, per_kt, pvdepth=None):
            """scores^T -> exp -> causal mask (narrow zone) -> PV into zp."""
            if pvdepth is None:
                pvdepth = PVQ
            zp = psz.tile([DH + 1, QT], F32)
            nkt = 4 * qt + 4
            blk = blk_serial[0]
            blk_serial[0] += 1

            def pv(kt, es, lo):
                ensure_tr(kt, h)
                nc.tensor.matmul(
                    zp[:, lo:QT],
                    lhsT=v_aug[:, h, kt, :],
                    rhs=es[:, lo:QT],
                    start=(kt == 0),
                    stop=(kt == nkt - 1),
                )

            for kt in range(nkt):
                rr = kt - 4 * qt
                lo = lo_of(rr)
                plo = pv_lo_of(rr)
                sp = ps.tile([128, QT], F32, tag="ps")
                nc.tensor.matmul(
                    sp[:, lo:QT],
                    lhsT=kh(h)[:, kt * 128 : (kt + 1) * 128],
                    rhs=qh(h)[:, qt * QT + lo : (qt + 1) * QT],
                    start=True,
                    stop=True,
                )
                es = expp.tile([128, QT], PV_DT, tag="expp")
                nc.scalar.activation(
                    out=es[:, plo:QT], in_=sp[:, plo:QT], func=AF.Exp
                )
                if rr >= 0:  # diagonal: zero where key > query
                    z0 = 128 * rr
                    zw = 128
                    nc.gpsimd.affine_select(
                        out=es[:, z0 : z0 + zw],
                        in_=es[:, z0 : z0 + zw],
                        compare_op=mybir.AluOpType.is_ge,
                        fill=0.0,
                        base=z0 - 128 * rr,
                        channel_multiplier=-1,
                        pattern=[[1, zw]],
                    )
                pvq.append((blk, lambda kt=kt, es=es, lo=plo: pv(kt, es, lo)))
                if per_kt in (2, 3, 4):
                    drain(per_kt)
                elif per_kt == 9:
                    drain(1)
                elif kt % 2 == 0:
                    drain(1)
                pv_drain(pvdepth)
            return zp, blk

        def normalize(zp, h, qt, cols=slice(0, QT)):
            rec = small.tile([1, QT], F32R, tag="rec")
            with nc.allow_low_precision(reason="f32r is fp32-precision"):
                nc.vector.reciprocal(rec[:, cols], zp[DH : DH + 1, cols])
            bc = ps.tile([128, QT], F32, tag="ps")
            nc.tensor.matmul(
                bc[0:64, cols], lhsT=ones64[:], rhs=rec[:, cols], start=True, stop=True
            )
            bc_sb = small.tile([64, QT], F32, tag="bcsb")
            if qt == NQT - 1:
                nc.vector.tensor_copy(out=bc_sb[:, cols], in_=bc[0:64, cols])
            else:
                nc.scalar.activation(out=bc_sb[:, cols], in_=bc[0:64, cols], func=AF.Copy)
            nc.vector.tensor_mul(
                zdst[h][:, qt * QT : (qt + 1) * QT][:, cols],
                zp[0:DH, cols],
                bc_sb[:, cols],
            )

        # ---- schedule ----
        # prologue: only what attention(h0, qt0) needs; the rest queues up.
        for mi in (0, 2, 1):
            for kpair in range(3):
                proj_unit(mi, 0, kpair)
        q_tr(range(4), pieces=(0,))
        q_proj(0, mis=(3,))
        q_tr(range(4), pieces=(1,))
        q_proj(0, mis=(4,))
        q_tr(range(4), pieces=(2,))
        for n in range(1, NQT):
            q_proj(n, mis=(0, 2, 1))
            q_tr(range(4 * n, 4 * n + 4), pieces=(0,))
            q_proj(n, mis=(3,))
            q_tr(range(4 * n, 4 * n + 4), pieces=(1,))
            q_proj(n, mis=(4,))
            q_tr(range(4 * n, 4 * n + 4), pieces=(2,))

        pending = None
        for qt in range(NQT):
            per_kt = [PACE0, 1, 1, PACE3][qt]
            for h in range(HPC):
                if pending is not None and NORM_EARLY:
                    # normalize the pending block BEFORE the next block's
                    # eviction burst so its reciprocal isn't queued behind
                    # them on DVE (the PE-side broadcast waits on it)
                    pv_flush(pending[3])
                    normalize(*pending[:3])
                    ph, pqt = pending[1], pending[2]
                    if ph == HPC - 1:
                        for t in range(4 * pqt, 4 * pqt + 4):
                            for n2 in range(2):
                                work.append(
                                    (("o", pqt), lambda t=t, n2=n2: o_proj_unit(t, n2))
                                )
                    pending = None
                force_drain_for(h, qt)
                zp, blk = attention(
                    h, qt, per_kt,
                    pvdepth=LPVQ if (qt == NQT - 1 and h == HPC - 1) else PVQ,
                )
                if pending is not None:
                    pv_flush(pending[3])  # pending block's PV accumulation done
                    normalize(*pending[:3])
                    ph, pqt = pending[1], pending[2]
                    if ph == HPC - 1:  # whole q-tile normalized -> O-proj ready
                        for t in range(4 * pqt, 4 * pqt + 4):
                            for n2 in range(2):
                                work.append(
                                    (("o", pqt), lambda t=t, n2=n2: o_proj_unit(t, n2))
                                )
                pending = (zp, h, qt, blk)
        # final block: normalize in column halves so the last O-proj pairs
        # start while the second half's recip/broadcast chain is still running
        pv_flush(pending[3])
        drain_all()
        if EPI == "quarter":
            for quarter in range(4):
                normalize(*pending[:3], cols=slice(quarter * 128, (quarter + 1) * 128))
                for n2 in range(2):
                    o_proj_unit(12 + quarter, n2, solo_dma=True)
        else:
            for half in range(2):
                normalize(*pending[:3], cols=slice(half * 256, (half + 1) * 256))
                for t in (12 + 2 * half, 13 + 2 * half):
                    for n2 in range(2):
                        o_proj_unit(t, n2, solo_dma=True)
    nc.finalize()
    return nc


_NC_CACHE = {}


def make_in_maps(x, W_qkv, b_qkv, W_o):
    in_maps = []
    for c in range(8):
        b, g = divmod(c, 4)
        hs = [HPC * g + i for i in range(HPC)]
        qr = [np.arange(64 * h, 64 * h + 64) for h in hs]
        w_q = [W_qkv[i] * 0.125 for i in qr]
        w_k = [W_qkv[768 + i] for i in qr]
        w_v = [W_qkv[1536 + i] for i in qr]
        b_q = [b_qkv[i] * 0.125 for i in qr]
        b_k = [b_qkv[768 + i] for i in qr]
        # packed rows: m0=[q0 q1] m1=[q2 v0] m2=[k0 k1] m3=[k2 v1] m4=[v2]
        wpk = np.concatenate(
            [w_q[0], w_q[1], w_q[2], w_v[0], w_k[0], w_k[1], w_k[2], w_v[1], w_v[2]],
            axis=0,
        )
        bqk_col = np.zeros((128, 4), np.float32)
        bqk_col[:, 0] = np.concatenate([b_q[0], b_q[1]])
        bqk_col[0:64, 1] = b_q[2]
        bqk_col[:, 2] = np.concatenate([b_k[0], b_k[1]])
        bqk_col[0:64, 3] = b_k[2]
        in_dt = ml_dtypes.bfloat16 if os.environ.get("K_BF16", "1") == "1" else np.float32
        in_maps.append(
            {
                "xT": np.ascontiguousarray(x[b].T).astype(in_dt),
                "wpk": np.ascontiguousarray(wpk.T).astype(in_dt),
                "woT": np.ascontiguousarray(W_o[:, GD * g : GD * (g + 1)].T),
                "bqk": bqk_col,
                "vones": np.ones((128, 64), np.float32),
            }
        )
    return in_maps


def make_in_maps_for_test(inputs):
    return make_in_maps(
        np.asarray(inputs["x"], np.float32),
        np.asarray(inputs["W_qkv"], np.float32),
        np.asarray(inputs["b_qkv"], np.float32),
        np.asarray(inputs["W_o"], np.float32),
    )


def kernel(x, W_qkv, b_qkv, W_o, b_o):
    x = np.asarray(x, np.float32)
    W_qkv = np.asarray(W_qkv, np.float32)
    b_qkv = np.asarray(b_qkv, np.float32)
    W_o = np.asarray(W_o, np.float32)
    b_o = np.asarray(b_o, np.float32)

    if "nc" not in _NC_CACHE:
        _NC_CACHE["nc"] = build_bass()
    nc = _NC_CACHE["nc"]

    in_maps = make_in_maps(x, W_qkv, b_qkv, W_o)

    res = run_bass_kernel_spmd(
        nc,
        in_maps,
        list(range(8)),
        trace=bool(int(os.environ.get("KERNEL_TRACE", "0"))),
    )
    _NC_CACHE["last_results"] = res

    out = np.zeros((B, S, D), np.float32)
    for c in range(8):
        out[c // 4] += np.asarray(res.results[c]["out_p"], np.float32)
    out += b_qkv[1536:] @ W_o.T + b_o
    return out


# revision 37
# speedup vs baseline: 1.1598x; 1.0672x over previous
"""Causal multi-head attention block (B=2, S=2048, D=768, H=12) on 8 trn2 cores.

Sharding: core c -> batch b = c//4 (data parallel), head group g = c%4
(tensor parallel, 3 heads per group). Each core computes its group's QKV
projection, causal attention, and a partial O-projection over its 192
z-columns. Host sums the 4 partials per batch and adds the biases that
commute through the math (v-bias and b_o).

On-core layout (everything "transposed", d on partitions, seq on free):
  xT   [768, 2048]   q/kT  [64*, 2048]      scores^T [keys, q]
so the softmax denominator comes free from a ones-column appended to V in
the PV matmul, and no on-chip transposes of activations are needed except
V (built via PE transpose from V^T).

The QKV projection uses a host-repacked weight matrix so every 128-wide
M-group is fully used:
  m0=[q_h0 q_h1] m1=[q_h2 v_h0] m2=[k_h0 k_h1] m3=[k_h2 v_h1] m4=[v_h2]
(q rows pre-scaled by 1/8; v bias folded into the host-side epilogue).

Matmul operands are float32r (full-rate fp32 on the PE). Scheduling
interleaves projection/transpose/O-proj work into the attention loop so
the scalar engine (exp) is never starved by a long PE FIFO stretch.

Cost-structure details:
  - fp32r matmuls under 256 output columns run at 1/4 rate, so the rr=3
    diagonal block is computed 256 wide (the causal mask zeroes the
    overhang) instead of 128 wide.
  - causal masks only touch the 128/256-wide zone that can violate
    causality instead of the whole remaining tile.
  - V^T and the identity live in f32r so the V transposes run at 1.5
    cycles/row instead of fp32's 2.0.
  - dummy PE transposes during the DMA prologue keep the tensor engine
    busy so the p-state ramp finishes before real matmuls arrive.
  - the last four token tiles store per 384-column half as soon as each
    O-proj eviction lands, shortening the end-of-kernel DMA tail.
"""

import os
from collections import deque
from contextlib import ExitStack

import numpy as np
import ml_dtypes

import concourse.tile as tile
from concourse import bacc, mybir
from concourse.bass_utils import run_bass_kernel_spmd

F32 = mybir.dt.float32
F32R = mybir.dt.float32r
AF = mybir.ActivationFunctionType

B, S, D = 2, 2048, 768
NH, DH = 12, 64
HPC = 3            # heads per core
GD = HPC * DH      # 192 z-cols per core
KT, QT = 128, 512  # key tile (partitions), q tile (psum free)
NKT, NQT = S // KT, S // QT   # 16, 4
NTOK = S // 128    # 16 token tiles
NKD = D // 128     # 6 contraction tiles for the projections
WPK = 2 * GD + GD  # 576 packed projection rows

N_WARMUP = int(os.environ.get("K_WARMUP", "16"))   # dummy PE transposes in prologue
PVQ = int(os.environ.get("K_PVQ", "7"))            # PV queue lag depth
LPVQ = int(os.environ.get("K_LPVQ", "7"))          # PV lag depth for the final block
FCOPY = os.environ.get("K_FCOPY", "mix")           # act|mix: final 4-tile eviction engines
NORM_EARLY = os.environ.get("K_NORME", "0") == "1" # normalize before next force_drain
PACE3 = int(os.environ.get("K_PACE3", "9"))        # qt3 drain mode
BFPV = os.environ.get("K_BFPV", "1") == "1"        # bf16 es/V path (PV + transposes)
BCPSUM = os.environ.get("K_BCPSUM", "1") == "1"    # multiply straight from bc PSUM
BF16_IN = os.environ.get("K_BF16", "1") == "1"     # load x / W_qkv in bf16
BF16_OUT = os.environ.get("K_BF16O", "1") == "1"   # store O-proj partials in bf16
EV_SPLIT = os.environ.get("K_EVSPLIT", "none")   # none|pool2|pool23: proj evictions offload
PACE0 = int(os.environ.get("K_PACE0", "2"))        # qt0 drain units per kt
EPI = os.environ.get("K_EPI", "half")           # half|quarter: final normalize granularity


def build_bass():
    nc = bacc.Bacc(None)
    in_dt_d = mybir.dt.bfloat16 if BF16_IN else F32
    xT = nc.dram_tensor("xT", [D, S], in_dt_d, kind="ExternalInput")
    wpk = nc.dram_tensor("wpk", [D, WPK], in_dt_d, kind="ExternalInput")
    woT = nc.dram_tensor("woT", [GD, D], F32, kind="ExternalInput")
    bqk = nc.dram_tensor("bqk", [128, 4], F32, kind="ExternalInput")
    out_dt_d = mybir.dt.bfloat16 if BF16_OUT else F32
    out_p = nc.dram_tensor("out_p", [S, D], out_dt_d, kind="ExternalOutput")

    with tile.TileContext(nc) as tc, ExitStack() as ctx:
        const = ctx.enter_context(tc.tile_pool(name="const", bufs=1))
        ps = ctx.enter_context(tc.tile_pool(name="ps", bufs=6, space="PSUM"))
        psz = ctx.enter_context(tc.tile_pool(name="psz", bufs=2, space="PSUM"))
        expp = ctx.enter_context(tc.tile_pool(name="expp", bufs=9))
        small = ctx.enter_context(tc.tile_pool(name="small", bufs=4))

        IN_DT = mybir.dt.bfloat16 if BF16_IN else F32R
        OUT_DT = mybir.dt.bfloat16 if BF16_OUT else F32
        xT_sb = const.tile([128, NKD, S], IN_DT)
        wpk_sb = const.tile([128, NKD, WPK], IN_DT)
        wo_a = const.tile([128, D], F32R)
        wo_b = const.tile([64, D], F32R)
        bqk_sb = const.tile([128, 4], F32)
        qT_sb = const.tile([128, 2, S], F32R)
        kT_sb = const.tile([128, 2, S], F32R)
        PV_DT = mybir.dt.bfloat16 if BFPV else F32R
        vvT = const.tile([128, 2, S], PV_DT)
        v_aug = const.tile([128, HPC, NKT, 2 * DH], PV_DT)
        zT01 = const.tile([128, S], F32R)
        zT2 = const.tile([64, S], F32R)
        ident = const.tile([128, 128], F32R)

        # warmup: keep the PE busy while the first DMAs land so the p-state
        # ramp is done before real matmuls issue. The transposes read the
        # not-yet-written identity tile; the values are irrelevant (nothing
        # reads `warm`) and the WAR ordering only delays make_identity to
        # ~1.5us, well before the first real V transpose needs it.
        warm = ps.tile([128, QT], F32R, tag="ps", name="warm")
        for _ in range(N_WARMUP):
            nc.tensor.transpose(warm[:, 0:128], ident[:, :], ident[:, :])

        # f32r identity: memset must run on an f32 view (Memset of f32r
        # fails the ISA check) while the affine_select writes the f32r view
        # so downstream f32r matmuls see properly rounded inputs.
        nc.gpsimd.memset(ident[:].bitcast(F32), 0.0)
        nc.gpsimd.affine_select(
            out=ident[:],
            in_=ident[:],
            compare_op=mybir.AluOpType.not_equal,
            fill=1.0,
            base=0,
            pattern=[[-1, 128]],
            channel_multiplier=1,
        )
        ident_pv = const.tile([128, 128], PV_DT)
        nc.vector.tensor_copy(out=ident_pv[:], in_=ident[:])
        # fill v_aug's 64 ones-columns with an always-false affine_select
        # (base=-1 never satisfies is_ge 0, so fill=1.0 lands everywhere);
        # keeps the fill off the DMA path and runs on the idle Pool engine
        nc.gpsimd.affine_select(
            out=v_aug[:, :, :, DH : 2 * DH],
            in_=v_aug[:, :, :, DH : 2 * DH],
            compare_op=mybir.AluOpType.is_ge,
            fill=1.0,
            base=-1,
            channel_multiplier=0,
            pattern=[[0, HPC], [0, NKT], [0, DH]],
        )

        # ---- loads. HWDGE costs ~625ns of descriptor generation per
        # dma_start regardless of size, so tiles are fetched in k-PAIRS for
        # the latency-critical prologue (projection k-steps consume pairs)
        # and in bigger merged transfers for everything later.
        xT_p = xT.rearrange("(t p) s -> p t s", p=128)
        wpk_p = wpk.rearrange("(t p) m -> p t m", p=128)
        for t0 in range(0, NKD, 2):
            nc.sync.dma_start(
                out=wpk_sb[:, t0 : t0 + 2, 0:384], in_=wpk_p[:, t0 : t0 + 2, 0:384]
            )
            nc.sync.dma_start(
                out=xT_sb[:, t0 : t0 + 2, 0:QT], in_=xT_p[:, t0 : t0 + 2, 0:QT]
            )
        nc.sync.dma_start(out=bqk_sb[:], in_=bqk[:, :])
        nc.sync.dma_start(out=wpk_sb[:, :, 384:WPK], in_=wpk_p[:, :, 384:WPK])
        for t0 in range(0, NKD, 3):
            nc.sync.dma_start(
                out=xT_sb[:, t0 : t0 + 3, QT : 2 * QT],
                in_=xT_p[:, t0 : t0 + 3, QT : 2 * QT],
            )
        nc.sync.dma_start(out=wo_a[:], in_=woT[0:128, :].bitcast(F32R))
        nc.sync.dma_start(out=wo_b[:], in_=woT[128:GD, :].bitcast(F32R))
        for t0 in range(0, NKD, 3):
            nc.sync.dma_start(
                out=xT_sb[:, t0 : t0 + 3, 2 * QT : S],
                in_=xT_p[:, t0 : t0 + 3, 2 * QT : S],
            )

        # packed projection m-groups: (col0, rows, evict spec)
        # evict spec: list of (psum row range, dst ap fn, bias col or None)
        def ev_q(col):
            return lambda n, r0, r1: qT_sb[r0:r1, col, n * QT : (n + 1) * QT]

        def ev_k(col):
            return lambda n, r0, r1: kT_sb[r0:r1, col, n * QT : (n + 1) * QT]

        def ev_v(col):
            return lambda n, r0, r1: vvT[r0:r1, col, n * QT : (n + 1) * QT]

        mgroups = [
            (0, 128, [((0, 128), ev_q(0), 0)]),
            (128, 128, [((0, 64), ev_q(1), 1), ((64, 128), ev_v(0), None)]),
            (256, 128, [((0, 128), ev_k(0), 2)]),
            (384, 128, [((0, 64), ev_k(1), 3), ((64, 128), ev_v(1), None)]),
            (512, 64, [((0, 64), ev_v(0), None)]),
        ]
        # v pieces: v_h0 -> vvT[64:128, 0], v_h1 -> vvT[64:128, 1],
        # v_h2 -> vvT[0:64, 0] (from the m4 group, psum rows 0:64)

        proj_psums = {}

        def proj_unit(mi, n, kpair, evict_act=False):
            """Two K-step matmuls of group (mi, n); evictions after the last."""
            c0, msz, evicts = mgroups[mi]
            key = (mi, n)
            if key not in proj_psums:
                proj_psums[key] = ps.tile([128, QT], F32, tag="ps", name="projp")
            p = proj_psums[key]
            for k in (2 * kpair, 2 * kpair + 1):
                nc.tensor.matmul(
                    p[:msz, :],
                    lhsT=wpk_sb[:, k, c0 : c0 + msz],
                    rhs=xT_sb[:, k, n * QT : (n + 1) * QT],
                    start=(k == 0),
                    stop=(k == NKD - 1),
                )
            if kpair == 2:
                del proj_psums[key]
                for (r0, r1), dst, bcol in evicts:
                    if mi == 4:
                        dst_ap = dst(n, 0, 64)  # v_h2 rows live at psum 0:64
                    else:
                        dst_ap = dst(n, r0, r1)
                    if bcol is None:
                        nc.vector.tensor_copy(out=dst_ap, in_=p[r0:r1, :])
                        continue
                    # NOTE: GPSIMD/Pool cannot read PSUM, so eviction
                    # offload is limited to ACT (Identity = copy + bias).
                    if (
                        (EV_SPLIT == "act0" and mi == 0)
                        or (EV_SPLIT == "act02" and mi in (0, 2))
                        or (EV_SPLIT == "actn1" and evict_act and mi in (0, 2))
                    ):
                        nc.scalar.activation(
                            out=dst_ap,
                            in_=p[r0:r1, :],
                            func=AF.Identity,
                            bias=bqk_sb[r0:r1, bcol : bcol + 1],
                        )
                    else:
                        nc.vector.tensor_scalar_add(
                            out=dst_ap,
                            in0=p[r0:r1, :],
                            scalar1=bqk_sb[r0:r1, bcol : bcol + 1],
                        )

        def transpose_unit(t, piece):
            """piece 0/1/2 = head 0/1/2; v_h0/v_h1 at vvT[64:128,0/1], v_h2 at vvT[0:64,0]."""
            if piece == 2:
                src = vvT[0:64, 0, t * 128 : (t + 1) * 128]
                idn = ident_pv[0:64, 0:64]
            else:
                src = vvT[64:128, piece, t * 128 : (t + 1) * 128]
                idn = ident_pv[64:128, 64:128]
            pt = ps.tile([128, QT], PV_DT, tag="ps")
            nc.tensor.transpose(pt[:, 0:64], src, idn)
            nc.vector.tensor_copy(v_aug[:, piece, t, 0:64], pt[:, 0:64])

        out_pair = out_p.rearrange("(tp a p) d -> tp p a d", a=2, p=128)
        out_sing = out_p.rearrange("(t p) d -> t p d", p=128)
        o_pairs = {}

        def o_proj_unit(t, n2, solo_dma=False):
            key = t // 2
            po = ps.tile([128, QT], F32, tag="ps")
            nc.tensor.matmul(
                po[:, 0:384],
                lhsT=zT01[:, t * 128 : (t + 1) * 128],
                rhs=wo_a[:, n2 * 384 : (n2 + 1) * 384],
                start=True,
                stop=False,
            )
            nc.tensor.matmul(
                po[:, 0:384],
                lhsT=zT2[:, t * 128 : (t + 1) * 128],
                rhs=wo_b[:, n2 * 384 : (n2 + 1) * 384],
                start=False,
                stop=True,
            )
            if solo_dma:
                # end of kernel: copies alternate ACT/DVE so the two halves
                # stage in parallel, then one per-tile store fires.
                ob = o_pairs.setdefault(
                    key, expp.tile([128, 2, D], OUT_DT, tag="osb", name="osb", bufs=2)
                )
                dst = ob[:, t % 2, n2 * 384 : (n2 + 1) * 384]
                if FCOPY == "act" or n2 == 0:
                    nc.scalar.activation(out=dst, in_=po[:, 0:384], func=AF.Copy)
                else:
                    nc.vector.tensor_copy(out=dst, in_=po[:, 0:384])
                if n2 == 1:
                    nc.sync.dma_start(out=out_sing[t], in_=ob[:, t % 2, :])
                    if t % 2 == 1:
                        del o_pairs[key]
                return
            if key not in o_pairs:
                o_pairs[key] = expp.tile([128, 2, D], OUT_DT, tag="osb", name="osb", bufs=2)
            ob = o_pairs[key]
            if t >= 12 and (t + n2) % 2 == 0:
                nc.scalar.activation(
                    out=ob[:, t % 2, n2 * 384 : (n2 + 1) * 384],
                    in_=po[:, 0:384],
                    func=AF.Copy,
                )
            else:
                nc.vector.tensor_copy(
                    out=ob[:, t % 2, n2 * 384 : (n2 + 1) * 384], in_=po[:, 0:384]
                )
            if t % 2 == 1 and n2 == 1:
                del o_pairs[key]
                nc.sync.dma_start(out=out_pair[key], in_=ob[:, :, :])

        # background work queue of (key, fn), drained between attention
        # iterations. Queue order is topological (a group's transposes come
        # after its evictions), so force-draining "through the last needed
        # unit" preserves all producer->consumer program ordering.
        work = deque()

        def q_proj(n, mis=range(5)):
            ea = n >= 1 and EV_SPLIT == "actn1"
            for mi in mis:
                for kpair in range(3):
                    work.append(
                        (
                            ("proj", n, mi),
                            lambda mi=mi, n=n, kp=kpair, ea=ea: proj_unit(
                                mi, n, kp, evict_act=ea
                            ),
                        )
                    )

        trq = deque()  # transpose units, pulled only by ensure_tr (PV time)

        def q_tr(ts, pieces=range(HPC)):
            for t in ts:
                for piece in pieces:
                    trq.append(
                        ((t, piece), lambda t=t, p=piece: transpose_unit(t, p))
                    )

        def drain(k=1):
            for _ in range(k):
                if work:
                    work.popleft()[1]()

        def drain_all():
            while work:
                work.popleft()[1]()
            while trq:
                trq.popleft()[1]()

        PROJ_GROUPS_FOR_HEAD = {0: (0, 1, 2), 1: (0, 2, 3), 2: (1, 3, 4)}

        def _drain_through(needed):
            last = -1
            for i, (key, _) in enumerate(work):
                if key in needed:
                    last = i
            for _ in range(last + 1):
                work.popleft()[1]()

        def force_drain_for(h, qt):
            """Emit queued units the SCORES of attention(h, qt) depend on.
            V transposes are pulled lazily by the PV closures instead."""
            needed = set()
            for n in range(qt + 1):
                for mi in PROJ_GROUPS_FOR_HEAD[h]:
                    needed.add(("proj", n, mi))
            _drain_through(needed)

        def ensure_tr(t, piece):
            last = -1
            for i, (key, _) in enumerate(trq):
                if key == (t, piece):
                    last = i
            for _ in range(last + 1):
                trq.popleft()[1]()

        def qh(h):
            m, off = divmod(h * 64, 128)
            return qT_sb[off : off + 64, m, :]

        def kh(h):
            m, off = divmod(h * 64, 128)
            return kT_sb[off : off + 64, m, :]

        zdst = [zT01[0:64, :], zT01[64:128, :], zT2[0:64, :]]

        # PV matmuls are pipelined ~4 iterations behind their exp across
        # block boundaries, so the in-order PE FIFO never waits on the
        # exp/mask chain, not even at the end of a block.
        pvq = deque()  # (block_serial, pv_closure)
        blk_serial = [0]

        def pv_drain(depth):
            while len(pvq) > depth:
                pvq.popleft()[1]()

        def pv_flush(upto_serial):
            while pvq and pvq[0][0] <= upto_serial:
                pvq.popleft()[1]()

        def lo_of(rr):
            # query-column start of the computed region for a key tile with
            # diagonal offset rr; rr=3 is held at 256 so the fp32r SCORES
            # matmul stays >=256 wide (narrower pays 4x in rate).
            if rr <= 0:
                return 0
            return 128 * rr if rr < 3 else 256

        def pv_lo_of(rr):
            # bf16 PV has no narrow-width penalty, so the rr=3 PV reads just
            # the 128 columns that survive the causal mask
            if BFPV and rr == 3:
                return 384
            return lo_of(rr)

        def attention(h, qt, per_kt, pvdepth=None):
            """scores^T -> exp -> causal mask (narrow zone) -> PV into zp."""
            if pvdepth is None:
                pvdepth = PVQ
            zp = psz.tile([128, QT], F32)
            nkt = 4 * qt + 4
            blk = blk_serial[0]
            blk_serial[0] += 1

            def pv(kt, es, lo):
                ensure_tr(kt, h)
                nc.tensor.matmul(
                    zp[:, lo:QT],
                    lhsT=v_aug[:, h, kt, :],
                    rhs=es[:, lo:QT],
                    start=(kt == 0),
                    stop=(kt == nkt - 1),
                )

            for kt in range(nkt):
                rr = kt - 4 * qt
                lo = lo_of(rr)
                plo = pv_lo_of(rr)
                sp = ps.tile([128, QT], F32, tag="ps")
                nc.tensor.matmul(
                    sp[:, lo:QT],
                    lhsT=kh(h)[:, kt * 128 : (kt + 1) * 128],
                    rhs=qh(h)[:, qt * QT + lo : (qt + 1) * QT],
                    start=True,
                    stop=True,
                )
                es = expp.tile([128, QT], PV_DT, tag="expp")
                nc.scalar.activation(
                    out=es[:, plo:QT], in_=sp[:, plo:QT], func=AF.Exp
                )
                if rr >= 0:  # diagonal: zero where key > query
                    z0 = 128 * rr
                    zw = 128
                    nc.gpsimd.affine_select(
                        out=es[:, z0 : z0 + zw],
                        in_=es[:, z0 : z0 + zw],
                        compare_op=mybir.AluOpType.is_ge,
                        fill=0.0,
                        base=z0 - 128 * rr,
                        channel_multiplier=-1,
                        pattern=[[1, zw]],
                    )
                pvq.append((blk, lambda kt=kt, es=es, lo=plo: pv(kt, es, lo)))
                if per_kt in (2, 3, 4):
                    drain(per_kt)
                elif per_kt == 9:
                    drain(1)
                elif kt % 2 == 0:
                    drain(1)
                pv_drain(pvdepth)
            return zp, blk

        def normalize(zp, h, qt, cols=slice(0, QT)):
            # PV's 64 ones-columns already broadcast the softmax denominator
            # into zp rows 64:127, so normalization is recip + one multiply
            rec = small.tile([64, QT], F32R, tag="rec")
            with nc.allow_low_precision(reason="f32r is fp32-precision"):
                nc.vector.reciprocal(rec[:, cols], zp[DH : 2 * DH, cols])
            nc.vector.tensor_mul(
                zdst[h][:, qt * QT : (qt + 1) * QT][:, cols],
                zp[0:DH, cols],
                rec[:, cols],
            )

        # ---- schedule ----
        # prologue: only what attention(h0, qt0) needs; the rest queues up.
        for mi in (0, 2, 1):
            for kpair in range(3):
                proj_unit(mi, 0, kpair)
        q_tr(range(4), pieces=(0,))
        q_proj(0, mis=(3,))
        q_tr(range(4), pieces=(1,))
        q_proj(0, mis=(4,))
        q_tr(range(4), pieces=(2,))
        for n in range(1, NQT):
            q_proj(n, mis=(0, 2, 1))
            q_tr(range(4 * n, 4 * n + 4), pieces=(0,))
            q_proj(n, mis=(3,))
            q_tr(range(4 * n, 4 * n + 4), pieces=(1,))
            q_proj(n, mis=(4,))
            q_tr(range(4 * n, 4 * n + 4), pieces=(2,))

        pending = None
        for qt in range(NQT):
            per_kt = [PACE0, 1, 1, PACE3][qt]
            for h in range(HPC):
                if pending is not None and NORM_EARLY:
                    # normalize the pending block BEFORE the next block's
                    # eviction burst so its reciprocal isn't queued behind
                    # them on DVE (the PE-side broadcast waits on it)
                    pv_flush(pending[3])
                    normalize(*pending[:3])
                    ph, pqt = pending[1], pending[2]
                    if ph == HPC - 1:
                        for t in range(4 * pqt, 4 * pqt + 4):
                            for n2 in range(2):
                                work.append(
                                    (("o", pqt), lambda t=t, n2=n2: o_proj_unit(t, n2))
                                )
                    pending = None
                force_drain_for(h, qt)
                zp, blk = attention(
                    h, qt, per_kt,
                    pvdepth=LPVQ if (qt == NQT - 1 and h == HPC - 1) else PVQ,
                )
                if pending is not None:
                    pv_flush(pending[3])  # pending block's PV accumulation done
                    normalize(*pending[:3])
                    ph, pqt = pending[1], pending[2]
                    if ph == HPC - 1:  # whole q-tile normalized -> O-proj ready
                        for t in range(4 * pqt, 4 * pqt + 4):
                            for n2 in range(2):
                                work.append(
                                    (("o", pqt), lambda t=t, n2=n2: o_proj_unit(t, n2))
                                )
                pending = (zp, h, qt, blk)
        # final block: normalize in column halves so the last O-proj pairs
        # start while the second half's recip/broadcast chain is still running
        pv_flush(pending[3])
        drain_all()
        if EPI == "quarter":
            for quarter in range(4):
                normalize(*pending[:3], cols=slice(quarter * 128, (quarter + 1) * 128))
                for n2 in range(2):
                    o_proj_unit(12 + quarter, n2, solo_dma=True)
        else:
            for half in range(2):
                normalize(*pending[:3], cols=slice(half * 256, (half + 1) * 256))
                for t in (12 + 2 * half, 13 + 2 * half):
                    for n2 in range(2):
                        o_proj_unit(t, n2, solo_dma=True)
    nc.finalize()
    return nc


_NC_CACHE = {}


def make_in_maps(x, W_qkv, b_qkv, W_o):
    in_maps = []
    for c in range(8):
        b, g = divmod(c, 4)
        hs = [HPC * g + i for i in range(HPC)]
        qr = [np.arange(64 * h, 64 * h + 64) for h in hs]
        w_q = [W_qkv[i] * 0.125 for i in qr]
        w_k = [W_qkv[768 + i] for i in qr]
        w_v = [W_qkv[1536 + i] for i in qr]
        b_q = [b_qkv[i] * 0.125 for i in qr]
        b_k = [b_qkv[768 + i] for i in qr]
        # packed rows: m0=[q0 q1] m1=[q2 v0] m2=[k0 k1] m3=[k2 v1] m4=[v2]
        wpk = np.concatenate(
            [w_q[0], w_q[1], w_q[2], w_v[0], w_k[0], w_k[1], w_k[2], w_v[1], w_v[2]],
            axis=0,
        )
        bqk_col = np.zeros((128, 4), np.float32)
        bqk_col[:, 0] = np.concatenate([b_q[0], b_q[1]])
        bqk_col[0:64, 1] = b_q[2]
        bqk_col[:, 2] = np.concatenate([b_k[0], b_k[1]])
        bqk_col[0:64, 3] = b_k[2]
        in_dt = ml_dtypes.bfloat16 if os.environ.get("K_BF16", "1") == "1" else np.float32
        in_maps.append(
            {
                "xT": np.ascontiguousarray(x[b].T).astype(in_dt),
                "wpk": np.ascontiguousarray(wpk.T).astype(in_dt),
                "woT": np.ascontiguousarray(W_o[:, GD * g : GD * (g + 1)].T),
                "bqk": bqk_col,
            }
        )
    return in_maps


def make_in_maps_for_test(inputs):
    return make_in_maps(
        np.asarray(inputs["x"], np.float32),
        np.asarray(inputs["W_qkv"], np.float32),
        np.asarray(inputs["b_qkv"], np.float32),
        np.asarray(inputs["W_o"], np.float32),
    )


def kernel(x, W_qkv, b_qkv, W_o, b_o):
    x = np.asarray(x, np.float32)
    W_qkv = np.asarray(W_qkv, np.float32)
    b_qkv = np.asarray(b_qkv, np.float32)
    W_o = np.asarray(W_o, np.float32)
    b_o = np.asarray(b_o, np.float32)

    if "nc" not in _NC_CACHE:
        _NC_CACHE["nc"] = build_bass()
    nc = _NC_CACHE["nc"]

    in_maps = make_in_maps(x, W_qkv, b_qkv, W_o)

    res = run_bass_kernel_spmd(
        nc,
        in_maps,
        list(range(8)),
        trace=bool(int(os.environ.get("KERNEL_TRACE", "0"))),
    )
    _NC_CACHE["last_results"] = res

    out = np.zeros((B, S, D), np.float32)
    for c in range(8):
        out[c // 4] += np.asarray(res.results[c]["out_p"], np.float32)
    out += b_qkv[1536:] @ W_o.T + b_o
    return out


# revision 38
# speedup vs baseline: 1.1874x; 1.0238x over previous
"""Causal multi-head attention block (B=2, S=2048, D=768, H=12) on 8 trn2 cores.

Sharding: core c -> batch b = c//4 (data parallel), head group g = c%4
(tensor parallel, 3 heads per group). Each core computes its group's QKV
projection, causal attention, and a partial O-projection over its 192
z-columns. Host sums the 4 partials per batch and adds the biases that
commute through the math (v-bias and b_o).

On-core layout (everything "transposed", d on partitions, seq on free):
  xT   [768, 2048]   q/kT  [64*, 2048]      scores^T [keys, q]
so the softmax denominator comes free from a ones-column appended to V in
the PV matmul, and no on-chip transposes of activations are needed except
V (built via PE transpose from V^T).

The QKV projection uses a host-repacked weight matrix so every 128-wide
M-group is fully used:
  m0=[q_h0 q_h1] m1=[q_h2 v_h0] m2=[k_h0 k_h1] m3=[k_h2 v_h1] m4=[v_h2]
(q rows pre-scaled by 1/8; v bias folded into the host-side epilogue).

Matmul operands are float32r (full-rate fp32 on the PE). Scheduling
interleaves projection/transpose/O-proj work into the attention loop so
the scalar engine (exp) is never starved by a long PE FIFO stretch.

Cost-structure details:
  - fp32r matmuls under 256 output columns run at 1/4 rate, so the rr=3
    diagonal block is computed 256 wide (the causal mask zeroes the
    overhang) instead of 128 wide.
  - causal masks only touch the 128/256-wide zone that can violate
    causality instead of the whole remaining tile.
  - V^T and the identity live in f32r so the V transposes run at 1.5
    cycles/row instead of fp32's 2.0.
  - dummy PE transposes during the DMA prologue keep the tensor engine
    busy so the p-state ramp finishes before real matmuls arrive.
  - the last four token tiles store per 384-column half as soon as each
    O-proj eviction lands, shortening the end-of-kernel DMA tail.
"""

import os
from collections import deque
from contextlib import ExitStack

import numpy as np
import ml_dtypes

import concourse.tile as tile
from concourse import bacc, mybir
from concourse.bass_utils import run_bass_kernel_spmd

F32 = mybir.dt.float32
F32R = mybir.dt.float32r
AF = mybir.ActivationFunctionType

B, S, D = 2, 2048, 768
NH, DH = 12, 64
HPC = 3            # heads per core
GD = HPC * DH      # 192 z-cols per core
KT, QT = 128, 512  # key tile (partitions), q tile (psum free)
NKT, NQT = S // KT, S // QT   # 16, 4
NTOK = S // 128    # 16 token tiles
NKD = D // 128     # 6 contraction tiles for the projections
WPK = 2 * GD + GD  # 576 packed projection rows

N_WARMUP = int(os.environ.get("K_WARMUP", "16"))   # dummy PE transposes in prologue
PVQ = int(os.environ.get("K_PVQ", "6"))            # PV queue lag depth
LPVQ = int(os.environ.get("K_LPVQ", "7"))          # PV lag depth for the final block
FCOPY = os.environ.get("K_FCOPY", "mix")           # act|mix: final 4-tile eviction engines
NORM_EARLY = os.environ.get("K_NORME", "0") == "1" # normalize before next force_drain
PACE3 = int(os.environ.get("K_PACE3", "9"))        # qt3 drain mode
BFPV = os.environ.get("K_BFPV", "1") == "1"        # bf16 es/V path (PV + transposes)
BCPSUM = os.environ.get("K_BCPSUM", "1") == "1"    # multiply straight from bc PSUM
BF16_IN = os.environ.get("K_BF16", "1") == "1"     # load x / W_qkv in bf16
BF16_OUT = os.environ.get("K_BF16O", "1") == "1"   # store O-proj partials in bf16
EV_SPLIT = os.environ.get("K_EVSPLIT", "none")   # none|pool2|pool23: proj evictions offload
PACE0 = int(os.environ.get("K_PACE0", "2"))        # qt0 drain units per kt
EPI = os.environ.get("K_EPI", "half")           # half|quarter: final normalize granularity


def build_bass():
    nc = bacc.Bacc(None)
    in_dt_d = mybir.dt.bfloat16 if BF16_IN else F32
    xT = nc.dram_tensor("xT", [D, S], in_dt_d, kind="ExternalInput")
    wpk = nc.dram_tensor("wpk", [D, WPK], in_dt_d, kind="ExternalInput")
    woT = nc.dram_tensor("woT", [GD, D], F32, kind="ExternalInput")
    bqk = nc.dram_tensor("bqk", [128, 4], F32, kind="ExternalInput")
    out_dt_d = mybir.dt.bfloat16 if BF16_OUT else F32
    out_p = nc.dram_tensor("out_p", [S, D], out_dt_d, kind="ExternalOutput")

    with tile.TileContext(nc) as tc, ExitStack() as ctx:
        const = ctx.enter_context(tc.tile_pool(name="const", bufs=1))
        ps = ctx.enter_context(tc.tile_pool(name="ps", bufs=6, space="PSUM"))
        psz = ctx.enter_context(tc.tile_pool(name="psz", bufs=2, space="PSUM"))
        expp = ctx.enter_context(tc.tile_pool(name="expp", bufs=9))
        small = ctx.enter_context(tc.tile_pool(name="small", bufs=4))

        IN_DT = mybir.dt.bfloat16 if BF16_IN else F32R
        OUT_DT = mybir.dt.bfloat16 if BF16_OUT else F32
        xT_sb = const.tile([128, NKD, S], IN_DT)
        wpk_sb = const.tile([128, NKD, WPK], IN_DT)
        wo_a = const.tile([128, D], F32R)
        wo_b = const.tile([64, D], F32R)
        bqk_sb = const.tile([128, 4], F32)
        qT_sb = const.tile([128, 2, S], F32R)
        kT_sb = const.tile([128, 2, S], F32R)
        PV_DT = mybir.dt.bfloat16 if BFPV else F32R
        vvT = const.tile([128, 2, S], PV_DT)
        v_aug = const.tile([128, HPC, NKT, 2 * DH], PV_DT)
        zT01 = const.tile([128, S], F32R)
        zT2 = const.tile([64, S], F32R)
        ident = const.tile([128, 128], F32R)

        # warmup: keep the PE busy while the first DMAs land so the p-state
        # ramp is done before real matmuls issue. The transposes read the
        # not-yet-written identity tile; the values are irrelevant (nothing
        # reads `warm`) and the WAR ordering only delays make_identity to
        # ~1.5us, well before the first real V transpose needs it.
        warm = ps.tile([128, QT], F32R, tag="ps", name="warm")
        for _ in range(N_WARMUP):
            nc.tensor.transpose(warm[:, 0:128], ident[:, :], ident[:, :])

        # f32r identity: memset must run on an f32 view (Memset of f32r
        # fails the ISA check) while the affine_select writes the f32r view
        # so downstream f32r matmuls see properly rounded inputs.
        nc.gpsimd.memset(ident[:].bitcast(F32), 0.0)
        nc.gpsimd.affine_select(
            out=ident[:],
            in_=ident[:],
            compare_op=mybir.AluOpType.not_equal,
            fill=1.0,
            base=0,
            pattern=[[-1, 128]],
            channel_multiplier=1,
        )
        ident_pv = const.tile([128, 128], PV_DT)
        nc.vector.tensor_copy(out=ident_pv[:], in_=ident[:])
        # fill v_aug's 64 ones-columns with an always-false affine_select
        # (base=-1 never satisfies is_ge 0, so fill=1.0 lands everywhere);
        # keeps the fill off the DMA path and runs on the idle Pool engine
        nc.gpsimd.affine_select(
            out=v_aug[:, :, :, DH : 2 * DH],
            in_=v_aug[:, :, :, DH : 2 * DH],
            compare_op=mybir.AluOpType.is_ge,
            fill=1.0,
            base=-1,
            channel_multiplier=0,
            pattern=[[0, HPC], [0, NKT], [0, DH]],
        )

        # ---- loads. HWDGE costs ~625ns of descriptor generation per
        # dma_start regardless of size, so tiles are fetched in k-PAIRS for
        # the latency-critical prologue (projection k-steps consume pairs)
        # and in bigger merged transfers for everything later.
        xT_p = xT.rearrange("(t p) s -> p t s", p=128)
        wpk_p = wpk.rearrange("(t p) m -> p t m", p=128)
        for t0 in range(0, NKD, 2):
            nc.sync.dma_start(
                out=wpk_sb[:, t0 : t0 + 2, 0:384], in_=wpk_p[:, t0 : t0 + 2, 0:384]
            )
            nc.sync.dma_start(
                out=xT_sb[:, t0 : t0 + 2, 0:QT], in_=xT_p[:, t0 : t0 + 2, 0:QT]
            )
        nc.sync.dma_start(out=bqk_sb[:], in_=bqk[:, :])
        nc.sync.dma_start(out=wpk_sb[:, :, 384:WPK], in_=wpk_p[:, :, 384:WPK])
        for t0 in range(0, NKD, 3):
            nc.sync.dma_start(
                out=xT_sb[:, t0 : t0 + 3, QT : 2 * QT],
                in_=xT_p[:, t0 : t0 + 3, QT : 2 * QT],
            )
        nc.sync.dma_start(out=wo_a[:], in_=woT[0:128, :].bitcast(F32R))
        nc.sync.dma_start(out=wo_b[:], in_=woT[128:GD, :].bitcast(F32R))
        for t0 in range(0, NKD, 3):
            nc.sync.dma_start(
                out=xT_sb[:, t0 : t0 + 3, 2 * QT : S],
                in_=xT_p[:, t0 : t0 + 3, 2 * QT : S],
            )

        # packed projection m-groups: (col0, rows, evict spec)
        # evict spec: list of (psum row range, dst ap fn, bias col or None)
        def ev_q(col):
            return lambda n, r0, r1: qT_sb[r0:r1, col, n * QT : (n + 1) * QT]

        def ev_k(col):
            return lambda n, r0, r1: kT_sb[r0:r1, col, n * QT : (n + 1) * QT]

        def ev_v(col):
            return lambda n, r0, r1: vvT[r0:r1, col, n * QT : (n + 1) * QT]

        mgroups = [
            (0, 128, [((0, 128), ev_q(0), 0)]),
            (128, 128, [((0, 64), ev_q(1), 1), ((64, 128), ev_v(0), None)]),
            (256, 128, [((0, 128), ev_k(0), 2)]),
            (384, 128, [((0, 64), ev_k(1), 3), ((64, 128), ev_v(1), None)]),
            (512, 64, [((0, 64), ev_v(0), None)]),
        ]
        # v pieces: v_h0 -> vvT[64:128, 0], v_h1 -> vvT[64:128, 1],
        # v_h2 -> vvT[0:64, 0] (from the m4 group, psum rows 0:64)

        proj_psums = {}

        def proj_unit(mi, n, kpair, evict_act=False):
            """Two K-step matmuls of group (mi, n); evictions after the last."""
            c0, msz, evicts = mgroups[mi]
            key = (mi, n)
            if key not in proj_psums:
                proj_psums[key] = ps.tile([128, QT], F32, tag="ps", name="projp")
            p = proj_psums[key]
            for k in (2 * kpair, 2 * kpair + 1):
                nc.tensor.matmul(
                    p[:msz, :],
                    lhsT=wpk_sb[:, k, c0 : c0 + msz],
                    rhs=xT_sb[:, k, n * QT : (n + 1) * QT],
                    start=(k == 0),
                    stop=(k == NKD - 1),
                )
            if kpair == 2:
                del proj_psums[key]
                for (r0, r1), dst, bcol in evicts:
                    if mi == 4:
                        dst_ap = dst(n, 0, 64)  # v_h2 rows live at psum 0:64
                    else:
                        dst_ap = dst(n, r0, r1)
                    if bcol is None:
                        nc.vector.tensor_copy(out=dst_ap, in_=p[r0:r1, :])
                        continue
                    # NOTE: GPSIMD/Pool cannot read PSUM, so eviction
                    # offload is limited to ACT (Identity = copy + bias).
                    if (
                        (EV_SPLIT == "act0" and mi == 0)
                        or (EV_SPLIT == "act02" and mi in (0, 2))
                        or (EV_SPLIT == "actn1" and evict_act and mi in (0, 2))
                    ):
                        nc.scalar.activation(
                            out=dst_ap,
                            in_=p[r0:r1, :],
                            func=AF.Identity,
                            bias=bqk_sb[r0:r1, bcol : bcol + 1],
                        )
                    else:
                        nc.vector.tensor_scalar_add(
                            out=dst_ap,
                            in0=p[r0:r1, :],
                            scalar1=bqk_sb[r0:r1, bcol : bcol + 1],
                        )

        def transpose_unit(t, piece):
            """piece 0/1/2 = head 0/1/2; v_h0/v_h1 at vvT[64:128,0/1], v_h2 at vvT[0:64,0]."""
            if piece == 2:
                src = vvT[0:64, 0, t * 128 : (t + 1) * 128]
                idn = ident_pv[0:64, 0:64]
            else:
                src = vvT[64:128, piece, t * 128 : (t + 1) * 128]
                idn = ident_pv[64:128, 64:128]
            pt = ps.tile([128, QT], PV_DT, tag="ps")
            nc.tensor.transpose(pt[:, 0:64], src, idn)
            nc.vector.tensor_copy(v_aug[:, piece, t, 0:64], pt[:, 0:64])

        out_pair = out_p.rearrange("(tp a p) d -> tp p a d", a=2, p=128)
        out_sing = out_p.rearrange("(t p) d -> t p d", p=128)
        o_pairs = {}

        def o_proj_unit(t, n2, solo_dma=False):
            key = t // 2
            po = ps.tile([128, QT], F32, tag="ps")
            nc.tensor.matmul(
                po[:, 0:384],
                lhsT=zT01[:, t * 128 : (t + 1) * 128],
                rhs=wo_a[:, n2 * 384 : (n2 + 1) * 384],
                start=True,
                stop=False,
            )
            nc.tensor.matmul(
                po[:, 0:384],
                lhsT=zT2[:, t * 128 : (t + 1) * 128],
                rhs=wo_b[:, n2 * 384 : (n2 + 1) * 384],
                start=False,
                stop=True,
            )
            if solo_dma:
                # end of kernel: copies alternate ACT/DVE so the two halves
                # stage in parallel, then one per-tile store fires.
                ob = o_pairs.setdefault(
                    key, expp.tile([128, 2, D], OUT_DT, tag="osb", name="osb", bufs=2)
                )
                dst = ob[:, t % 2, n2 * 384 : (n2 + 1) * 384]
                if FCOPY == "act" or n2 == 0:
                    nc.scalar.activation(out=dst, in_=po[:, 0:384], func=AF.Copy)
                else:
                    nc.vector.tensor_copy(out=dst, in_=po[:, 0:384])
                if n2 == 1:
                    nc.sync.dma_start(out=out_sing[t], in_=ob[:, t % 2, :])
                    if t % 2 == 1:
                        del o_pairs[key]
                return
            if key not in o_pairs:
                o_pairs[key] = expp.tile([128, 2, D], OUT_DT, tag="osb", name="osb", bufs=2)
            ob = o_pairs[key]
            if t >= 12 and (t + n2) % 2 == 0:
                nc.scalar.activation(
                    out=ob[:, t % 2, n2 * 384 : (n2 + 1) * 384],
                    in_=po[:, 0:384],
                    func=AF.Copy,
                )
            else:
                nc.vector.tensor_copy(
                    out=ob[:, t % 2, n2 * 384 : (n2 + 1) * 384], in_=po[:, 0:384]
                )
            if t % 2 == 1 and n2 == 1:
                del o_pairs[key]
                nc.sync.dma_start(out=out_pair[key], in_=ob[:, :, :])

        # background work queue of (key, fn), drained between attention
        # iterations. Queue order is topological (a group's transposes come
        # after its evictions), so force-draining "through the last needed
        # unit" preserves all producer->consumer program ordering.
        work = deque()

        def q_proj(n, mis=range(5)):
            ea = n >= 1 and EV_SPLIT == "actn1"
            for mi in mis:
                for kpair in range(3):
                    work.append(
                        (
                            ("proj", n, mi),
                            lambda mi=mi, n=n, kp=kpair, ea=ea: proj_unit(
                                mi, n, kp, evict_act=ea
                            ),
                        )
                    )

        trq = deque()  # transpose units, pulled only by ensure_tr (PV time)

        def q_tr(ts, pieces=range(HPC)):
            for t in ts:
                for piece in pieces:
                    trq.append(
                        ((t, piece), lambda t=t, p=piece: transpose_unit(t, p))
                    )

        def drain(k=1):
            for _ in range(k):
                if work:
                    work.popleft()[1]()

        def drain_all():
            while work:
                work.popleft()[1]()
            while trq:
                trq.popleft()[1]()

        PROJ_GROUPS_FOR_HEAD = {0: (0, 1, 2), 1: (0, 2, 3), 2: (1, 3, 4)}

        def _drain_through(needed):
            last = -1
            for i, (key, _) in enumerate(work):
                if key in needed:
                    last = i
            for _ in range(last + 1):
                work.popleft()[1]()

        def force_drain_for(h, qt):
            """Emit queued units the SCORES of attention(h, qt) depend on.
            V transposes are pulled lazily by the PV closures instead."""
            needed = set()
            for n in range(qt + 1):
                for mi in PROJ_GROUPS_FOR_HEAD[h]:
                    needed.add(("proj", n, mi))
            _drain_through(needed)

        def ensure_tr(t, piece):
            last = -1
            for i, (key, _) in enumerate(trq):
                if key == (t, piece):
                    last = i
            for _ in range(last + 1):
                trq.popleft()[1]()

        def qh(h):
            m, off = divmod(h * 64, 128)
            return qT_sb[off : off + 64, m, :]

        def kh(h):
            m, off = divmod(h * 64, 128)
            return kT_sb[off : off + 64, m, :]

        zdst = [zT01[0:64, :], zT01[64:128, :], zT2[0:64, :]]

        # PV matmuls are pipelined ~4 iterations behind their exp across
        # block boundaries, so the in-order PE FIFO never waits on the
        # exp/mask chain, not even at the end of a block.
        pvq = deque()  # (block_serial, pv_closure)
        blk_serial = [0]

        def pv_drain(depth):
            while len(pvq) > depth:
                pvq.popleft()[1]()

        def pv_flush(upto_serial):
            while pvq and pvq[0][0] <= upto_serial:
                pvq.popleft()[1]()

        def lo_of(rr):
            # query-column start of the computed region for a key tile with
            # diagonal offset rr; rr=3 is held at 256 so the fp32r SCORES
            # matmul stays >=256 wide (narrower pays 4x in rate).
            if rr <= 0:
                return 0
            return 128 * rr if rr < 3 else 256

        def pv_lo_of(rr):
            # bf16 PV has no narrow-width penalty, so the rr=3 PV reads just
            # the 128 columns that survive the causal mask
            if BFPV and rr == 3:
                return 384
            return lo_of(rr)

        def attention(h, qt, per_kt, pvdepth=None):
            """scores^T -> exp -> causal mask (narrow zone) -> PV into zp."""
            if pvdepth is None:
                pvdepth = PVQ
            zp = psz.tile([128, QT], F32)
            nkt = 4 * qt + 4
            blk = blk_serial[0]
            blk_serial[0] += 1

            def pv(kt, es, lo):
                ensure_tr(kt, h)
                nc.tensor.matmul(
                    zp[:, lo:QT],
                    lhsT=v_aug[:, h, kt, :],
                    rhs=es[:, lo:QT],
                    start=(kt == 0),
                    stop=(kt == nkt - 1),
                )

            for kt in range(nkt):
                rr = kt - 4 * qt
                lo = lo_of(rr)
                plo = pv_lo_of(rr)
                sp = ps.tile([128, QT], F32, tag="ps")
                nc.tensor.matmul(
                    sp[:, lo:QT],
                    lhsT=kh(h)[:, kt * 128 : (kt + 1) * 128],
                    rhs=qh(h)[:, qt * QT + lo : (qt + 1) * QT],
                    start=True,
                    stop=True,
                )
                es = expp.tile([128, QT], PV_DT, tag="expp")
                nc.scalar.activation(
                    out=es[:, plo:QT], in_=sp[:, plo:QT], func=AF.Exp
                )
                if rr >= 0:  # diagonal: zero where key > query
                    z0 = 128 * rr
                    zw = 128
                    nc.gpsimd.affine_select(
                        out=es[:, z0 : z0 + zw],
                        in_=es[:, z0 : z0 + zw],
                        compare_op=mybir.AluOpType.is_ge,
                        fill=0.0,
                        base=z0 - 128 * rr,
                        channel_multiplier=-1,
                        pattern=[[1, zw]],
                    )
                pvq.append((blk, lambda kt=kt, es=es, lo=plo: pv(kt, es, lo)))
                if per_kt in (2, 3, 4):
                    drain(per_kt)
                elif per_kt == 9:
                    drain(1)
                elif kt % 2 == 0:
                    drain(1)
                pv_drain(pvdepth)
            return zp, blk

        def normalize(zp, h, qt, cols=slice(0, QT)):
            # PV's 64 ones-columns already broadcast the softmax denominator
            # into zp rows 64:127, so normalization is recip + one multiply
            rec = small.tile([64, QT], F32R, tag="rec")
            with nc.allow_low_precision(reason="f32r is fp32-precision"):
                nc.vector.reciprocal(rec[:, cols], zp[DH : 2 * DH, cols])
            nc.vector.tensor_mul(
                zdst[h][:, qt * QT : (qt + 1) * QT][:, cols],
                zp[0:DH, cols],
                rec[:, cols],
            )

        # ---- schedule ----
        # prologue: only what attention(h0, qt0) needs; the rest queues up.
        for mi in (0, 2, 1):
            for kpair in range(3):
                proj_unit(mi, 0, kpair)
        q_tr(range(4), pieces=(0,))
        q_proj(0, mis=(3,))
        q_tr(range(4), pieces=(1,))
        q_proj(0, mis=(4,))
        q_tr(range(4), pieces=(2,))
        for n in range(1, NQT):
            q_proj(n, mis=(0, 2, 1))
            q_tr(range(4 * n, 4 * n + 4), pieces=(0,))
            q_proj(n, mis=(3,))
            q_tr(range(4 * n, 4 * n + 4), pieces=(1,))
            q_proj(n, mis=(4,))
            q_tr(range(4 * n, 4 * n + 4), pieces=(2,))

        pending = None
        for qt in range(NQT):
            per_kt = [PACE0, 1, 1, PACE3][qt]
            for h in range(HPC):
                if pending is not None and NORM_EARLY:
                    # normalize the pending block BEFORE the next block's
                    # eviction burst so its reciprocal isn't queued behind
                    # them on DVE (the PE-side broadcast waits on it)
                    pv_flush(pending[3])
                    normalize(*pending[:3])
                    ph, pqt = pending[1], pending[2]
                    if ph == HPC - 1:
                        for t in range(4 * pqt, 4 * pqt + 4):
                            for n2 in range(2):
                                work.append(
                                    (("o", pqt), lambda t=t, n2=n2: o_proj_unit(t, n2))
                                )
                    pending = None
                force_drain_for(h, qt)
                zp, blk = attention(
                    h, qt, per_kt,
                    pvdepth=LPVQ if (qt == NQT - 1 and h == HPC - 1) else PVQ,
                )
                if pending is not None:
                    pv_flush(pending[3])  # pending block's PV accumulation done
                    normalize(*pending[:3])
                    ph, pqt = pending[1], pending[2]
                    if ph == HPC - 1:  # whole q-tile normalized -> O-proj ready
                        for t in range(4 * pqt, 4 * pqt + 4):
                            for n2 in range(2):
                                work.append(
                                    (("o", pqt), lambda t=t, n2=n2: o_proj_unit(t, n2))
                                )
                pending = (zp, h, qt, blk)
        # final block: normalize in column halves so the last O-proj pairs
        # start while the second half's recip/broadcast chain is still running
        pv_flush(pending[3])
        drain_all()
        if EPI == "quarter":
            for quarter in range(4):
                normalize(*pending[:3], cols=slice(quarter * 128, (quarter + 1) * 128))
                for n2 in range(2):
                    o_proj_unit(12 + quarter, n2, solo_dma=True)
        else:
            for half in range(2):
                normalize(*pending[:3], cols=slice(half * 256, (half + 1) * 256))
                for t in (12 + 2 * half, 13 + 2 * half):
                    for n2 in range(2):
                        o_proj_unit(t, n2, solo_dma=True)
    nc.finalize()
    return nc


_NC_CACHE = {}


def make_in_maps(x, W_qkv, b_qkv, W_o):
    in_maps = []
    for c in range(8):
        b, g = divmod(c, 4)
        hs = [HPC * g + i for i in range(HPC)]
        qr = [np.arange(64 * h, 64 * h + 64) for h in hs]
        w_q = [W_qkv[i] * 0.125 for i in qr]
        w_k = [W_qkv[768 + i] for i in qr]
        w_v = [W_qkv[1536 + i] for i in qr]
        b_q = [b_qkv[i] * 0.125 for i in qr]
        b_k = [b_qkv[768 + i] for i in qr]
        # packed rows: m0=[q0 q1] m1=[q2 v0] m2=[k0 k1] m3=[k2 v1] m4=[v2]
        wpk = np.concatenate(
            [w_q[0], w_q[1], w_q[2], w_v[0], w_k[0], w_k[1], w_k[2], w_v[1], w_v[2]],
            axis=0,
        )
        bqk_col = np.zeros((128, 4), np.float32)
        bqk_col[:, 0] = np.concatenate([b_q[0], b_q[1]])
        bqk_col[0:64, 1] = b_q[2]
        bqk_col[:, 2] = np.concatenate([b_k[0], b_k[1]])
        bqk_col[0:64, 3] = b_k[2]
        in_dt = ml_dtypes.bfloat16 if os.environ.get("K_BF16", "1") == "1" else np.float32
        in_maps.append(
            {
                "xT": np.ascontiguousarray(x[b].T).astype(in_dt),
                "wpk": np.ascontiguousarray(wpk.T).astype(in_dt),
                "woT": np.ascontiguousarray(W_o[:, GD * g : GD * (g + 1)].T),
                "bqk": bqk_col,
            }
        )
    return in_maps


def make_in_maps_for_test(inputs):
    return make_in_maps(
        np.asarray(inputs["x"], np.float32),
        np.asarray(inputs["W_qkv"], np.float32),
        np.asarray(inputs["b_qkv"], np.float32),
        np.asarray(inputs["W_o"], np.float32),
    )


def kernel(x, W_qkv, b_qkv, W_o, b_o):
    x = np.asarray(x, np.float32)
    W_qkv = np.asarray(W_qkv, np.float32)
    b_qkv = np.asarray(b_qkv, np.float32)
    W_o = np.asarray(W_o, np.float32)
    b_o = np.asarray(b_o, np.float32)

    if "nc" not in _NC_CACHE:
        _NC_CACHE["nc"] = build_bass()
    nc = _NC_CACHE["nc"]

    in_maps = make_in_maps(x, W_qkv, b_qkv, W_o)

    res = run_bass_kernel_spmd(
        nc,
        in_maps,
        list(range(8)),
        trace=bool(int(os.environ.get("KERNEL_TRACE", "0"))),
    )
    _NC_CACHE["last_results"] = res

    out = np.zeros((B, S, D), np.float32)
    for c in range(8):
        out[c // 4] += np.asarray(res.results[c]["out_p"], np.float32)
    out += b_qkv[1536:] @ W_o.T + b_o
    return out
